# revision 20
# baseline (speedup 1.0000x reference)
"""Causal self-attention (B=4, T=2048, C=1024, H=16, Dh=64) on 8 trn2 NeuronCores.

Sharding: core i <-> (batch b = i//2, head-group g = i%2). Each core computes
8 heads of one batch end-to-end (qkv slice, causal attention, partial output
projection); the host sums the head-group/pair-couple partials per batch and
adds bproj. No device collectives.

All matmuls run as float32r (single-pass reduced-precision fp32 on the PE,
full-rate at moving-dim >= 256), accumulating in fp32 PSUM. Attention uses
the transposed-scores layout sT[tk, tq] so no per-block transposes are
needed: softmax denominators come out of the PV matmul via an extra ones
column interleaved into Wv, and are broadcast across partitions with a
partition-step-0 SBUF->SBUF DMA.
"""

import numpy as np

import concourse.bass as bass
import concourse.tile as tile
from concourse import bacc, mybir
from concourse.bass_utils import run_bass_kernel_spmd
from concourse.masks import make_identity

F32 = mybir.dt.float32
F32R = mybir.dt.float32r

N_CORES = 8
B, T, C = 4, 2048, 1024
NH_TOT, D = 16, 64
F = 512            # features per core (8 heads)
NH = 8             # local heads
NPAIR = 4          # head pairs (128 feats each)
CCH = C // 128     # 8 contraction chunks
NTT = T // 128     # 16 t tiles
NTB = T // 512     # 4 t blocks (qkv production)
NQB = T // 512     # 4 q blocks (attention)
VW = NH * (D + 1)  # 520: augmented v width
ADD = mybir.AluOpType.add
MULT = mybir.AluOpType.mult


def _emit(tc, aps):
    from contextlib import ExitStack
    nc = tc.nc
    x, wq, wk, wva, bq, bk, bva, wp = (
        aps["x"], aps["wq"], aps["wk"], aps["wva"], aps["bq"], aps["bk"],
        aps["bva"], aps["wp"])
    cmask = aps["cmask"]
    out_ab = [aps["out_pa"], aps["out_pb"]]

    # ---- pools (all coexist; ~210KB/partition total) ----
    ctx = ExitStack()
    pp_qk = ctx.enter_context(tc.tile_pool(name="ps_qk", bufs=2, space="PSUM"))
    pp_s = ctx.enter_context(tc.tile_pool(name="ps_s", bufs=2, space="PSUM"))
    pp_pv = ctx.enter_context(tc.tile_pool(name="ps_pv", bufs=2, space="PSUM"))
    po_v = ctx.enter_context(tc.tile_pool(name="v_all", bufs=1))
    po_mask = ctx.enter_context(tc.tile_pool(name="mask", bufs=1))
    po_wv = ctx.enter_context(tc.tile_pool(name="wv", bufs=8))
    po_qkt = ctx.enter_context(tc.tile_pool(name="qkT", bufs=2))
    po_bias = ctx.enter_context(tc.tile_pool(name="bias", bufs=1))
    po_xin = ctx.enter_context(tc.tile_pool(name="xin", bufs=2))
    po_xt = ctx.enter_context(tc.tile_pool(name="xT", bufs=1))
    po_wqk = ctx.enter_context(tc.tile_pool(name="wqk", bufs=8))
    po_yt = ctx.enter_context(tc.tile_pool(name="yT", bufs=2))
    po_exp = ctx.enter_context(tc.tile_pool(name="expT", bufs=2))
    po_rec = ctx.enter_context(tc.tile_pool(name="recip", bufs=2))
    po_den = ctx.enter_context(tc.tile_pool(name="den", bufs=1))
    po_ytmp = ctx.enter_context(tc.tile_pool(name="ytmp", bufs=1))
    po_wp = ctx.enter_context(tc.tile_pool(name="wp", bufs=4))
    po_out = ctx.enter_context(tc.tile_pool(name="out_sb", bufs=2))

    mask_sb = po_mask.tile([128, 1024], F32, tag="mask")
    nc.sync.dma_start(out=mask_sb[:], in_=cmask[:])
    bva_sb = po_bias.tile([1, VW], F32R, tag="bva")
    nc.sync.dma_start(out=bva_sb[:], in_=bva[:])
    ones_f32 = po_bias.tile([128, 128], F32, tag="ones_f32")
    nc.vector.memset(ones_f32[:], 1.0)
    ones_row = po_bias.tile([1, 128], F32R, tag="ones")
    nc.vector.tensor_copy(ones_row[:], ones_f32[0:1, :])
    ones64 = po_bias.tile([128, 64], F32R, tag="ones64")
    nc.vector.tensor_copy(ones64[:], ones_f32[:, 0:64])
    ident = po_bias.tile([128, 128], F32, tag="ident")
    make_identity(nc, ident[:])

    # ---- phase 0: x -> xT via PE transposes ----
    xT = [po_xt.tile([128, T], F32R, tag=f"xT{c}", name=f"xT{c}")
          for c in range(CCH)]
    for tt in range(NTT):
        xt_in = po_xin.tile([128, C], F32, tag="xin")
        nc.sync.dma_start(out=xt_in[:], in_=x[tt * 128:(tt + 1) * 128, :])
        tsl = slice(tt * 128, (tt + 1) * 128)
        for ca in range(2):
            pst = pp_qk.tile([128, 512], F32, tag="qk")
            for j in range(4):
                c = ca * 4 + j
                nc.tensor.transpose(
                    pst[:, j * 128:(j + 1) * 128],
                    xt_in[:, c * 128:(c + 1) * 128],
                    ident[:])
            for j in range(4):
                nc.vector.tensor_copy(
                    xT[ca * 4 + j][:, tsl], pst[:, j * 128:(j + 1) * 128])

    # ---- phase 0b: v (augmented with ones columns, all 8 heads) ----
    v_all = [po_v.tile([128, VW], F32R, tag=f"v{tt}", name=f"v{tt}")
             for tt in range(NTT)]
    for half in range(2):
        cs = slice(half * 260, half * 260 + 260)
        wv_sb = []
        for c in range(CCH):
            wt = po_wv.tile([128, 260], F32R, tag="wv")
            nc.sync.dma_start(out=wt[:], in_=wva[c * 128:(c + 1) * 128, cs])
            wv_sb.append(wt)
        for tt in range(NTT):
            ps = pp_qk.tile([128, 260], F32, tag="qk")
            for c in range(CCH):
                nc.tensor.matmul(
                    ps[:], xT[c][:, tt * 128:(tt + 1) * 128],
                    wv_sb[c][:], start=(c == 0), stop=False)
            nc.tensor.matmul(ps[:], ones_row[:], bva_sb[:, cs],
                             start=False, stop=True)
            nc.vector.tensor_copy(v_all[tt][:, cs], ps[:])

    # ---- per head pair: qkv -> attention -> partial proj ----
    yt_couple = []
    for pair in range(NPAIR):
        psl = slice(pair * 128, (pair + 1) * 128)

        # qT / kT for this pair
        wqk_c = []
        for c in range(CCH):
            wt = po_wqk.tile([128, 256], F32R, tag="wqk")
            nc.sync.dma_start(out=wt[:, 0:128],
                              in_=wq[c * 128:(c + 1) * 128, psl])
            nc.sync.dma_start(out=wt[:, 128:256],
                              in_=wk[c * 128:(c + 1) * 128, psl])
            wqk_c.append(wt)
        bq_sb = po_bias.tile([128, 1], F32, tag=f"bq{pair}", name=f"bq{pair}")
        nc.sync.dma_start(out=bq_sb[:], in_=bq[psl, :])
        bk_sb = po_bias.tile([128, 1], F32, tag=f"bk{pair}", name=f"bk{pair}")
        nc.sync.dma_start(out=bk_sb[:], in_=bk[psl, :])

        qT = po_qkt.tile([128, T], F32R, tag="qT")
        kT = po_qkt.tile([128, T], F32R, tag="kT")
        for tb in range(NTB):
            tsl = slice(tb * 512, (tb + 1) * 512)
            psq = pp_qk.tile([128, 512], F32, tag="qk")
            for c in range(CCH):
                nc.tensor.matmul(psq[:], wqk_c[c][:, 0:128],
                                 xT[c][:, tsl],
                                 start=(c == 0), stop=(c == CCH - 1))
            # (x@wq + bq) * 1/sqrt(D)
            nc.vector.tensor_scalar(
                out=qT[:, tsl], in0=psq[:], scalar1=bq_sb[:],
                scalar2=0.125, op0=ADD, op1=MULT)
            psk = pp_qk.tile([128, 512], F32, tag="qk")
            for c in range(CCH):
                nc.tensor.matmul(psk[:], wqk_c[c][:, 128:256],
                                 xT[c][:, tsl],
                                 start=(c == 0), stop=(c == CCH - 1))
            nc.vector.tensor_scalar(
                out=kT[:, tsl], in0=psk[:], scalar1=bk_sb[:],
                scalar2=None, op0=ADD)

        # attention for the pair's two heads
        yt = po_yt.tile([128, T], F32R, tag="yT")
        for hl in range(2):
            h = pair * 2 + hl
            rq = slice(hl * 64, hl * 64 + 64)
            vsl = slice(h * 65, h * 65 + 65)
            for qb in range(NQB):
                qsl = slice(qb * 512, (qb + 1) * 512)
                nkt = 4 * qb + 4
                pv = pp_pv.tile([128, 512], F32, tag="pv")
                for g0 in range(0, nkt, 2):
                    gs = min(2, nkt - g0)
                    st = pp_s.tile([128, 1024], F32, tag="s")
                    for i in range(gs):
                        kt = g0 + i
                        ssl = slice(i * 512, i * 512 + 512)
                        nc.tensor.matmul(
                            st[:, ssl],
                            kT[rq, kt * 128:(kt + 1) * 128],
                            qT[rq, qsl], start=True, stop=True)
                        j = kt - 4 * qb
                        if j >= 0:  # diagonal tile: additive causal mask
                            moff = 384 - 128 * j
                            nc.vector.tensor_add(
                                st[:, ssl], st[:, ssl],
                                mask_sb[:, moff:moff + 512])
                    et = po_exp.tile([128, 1024], F32R, tag="expT")
                    nc.scalar.activation(
                        et[:, 0:gs * 512], st[:, 0:gs * 512],
                        mybir.ActivationFunctionType.Exp)
                    for i in range(gs):
                        kt = g0 + i
                        nc.tensor.matmul(
                            pv[0:65, :], v_all[kt][:, vsl],
                            et[:, i * 512:(i + 1) * 512],
                            start=(kt == 0), stop=(kt == nkt - 1))
                # normalization: den row 64 -> broadcast -> recip -> mul
                # den row 64 -> (PE outer with ones) broadcast -> recip
                den = po_den.tile([128, 512], F32R, tag="den")
                nc.vector.tensor_copy(den[64:65, :], pv[64:65, :])
                bcp = pp_qk.tile([128, 512], F32, tag="qk")
                nc.tensor.matmul(bcp[0:64, :], ones64[64:65, :],
                                 den[64:65, :], start=True, stop=True)
                rec = po_rec.tile([128, 512], F32, tag="recip")
                nc.vector.reciprocal(rec[0:64, :], bcp[0:64, :])
                if hl == 0:
                    nc.vector.tensor_mul(yt[0:64, qsl], pv[0:64, :],
                                         rec[0:64, :])
                else:
                    # y must land on partitions 64..127 of the pair tile;
                    # engines can't cross partitions, so bounce via DMA.
                    ytmp = po_ytmp.tile([128, 512], F32R, tag="ytmp")
                    nc.vector.tensor_mul(ytmp[0:64, :], pv[0:64, :],
                                         rec[0:64, :])
                    nc.sync.dma_start(out=yt[64:128, qsl],
                                      in_=ytmp[0:64, :])
        yt_couple.append(yt)

        # partial projection per pair-couple (pairs 0+1 -> out_pa, 2+3 -> out_pb)
        if pair % 2 == 1:
            out_p = out_ab[pair // 2]
            wp_sb = []
            for pq in range(2):
                for cb in range(2):
                    prow = (pair - 1 + pq) * 128
                    wt = po_wp.tile([128, 512], F32R, tag="wp")
                    nc.sync.dma_start(
                        out=wt[:],
                        in_=wp[prow:prow + 128, cb * 512:(cb + 1) * 512])
                    wp_sb.append(wt)
            for tt in range(NTT):
                for cb in range(2):
                    ps = pp_qk.tile([128, 512], F32, tag="qk")
                    for pq in range(2):
                        nc.tensor.matmul(
                            ps[:],
                            yt_couple[pq][:, tt * 128:(tt + 1) * 128],
                            wp_sb[pq * 2 + cb][:],
                            start=(pq == 0), stop=(pq == 1))
                    ot = po_out.tile([128, 512], F32, tag="out")
                    nc.vector.tensor_copy(ot[:], ps[:])
                    nc.sync.dma_start(
                        out=out_p[tt * 128:(tt + 1) * 128,
                                  cb * 512:(cb + 1) * 512],
                        in_=ot[:])
            yt_couple = []
    ctx.close()


_CACHE = {}


def _build():
    if "nc" in _CACHE:
        return _CACHE["nc"]
    nc = bacc.Bacc("TRN2", target_bir_lowering=False, debug=False,
                   enable_asserts=True, num_devices=N_CORES)
    aps = {
        "x": nc.dram_tensor("x", [T, C], F32, kind="ExternalInput").ap(),
        "wq": nc.dram_tensor("wq", [C, F], F32R, kind="ExternalInput").ap(),
        "wk": nc.dram_tensor("wk", [C, F], F32R, kind="ExternalInput").ap(),
        "wva": nc.dram_tensor("wva", [C, VW], F32R, kind="ExternalInput").ap(),
        "bq": nc.dram_tensor("bq", [F, 1], F32, kind="ExternalInput").ap(),
        "bk": nc.dram_tensor("bk", [F, 1], F32, kind="ExternalInput").ap(),
        "bva": nc.dram_tensor("bva", [1, VW], F32R, kind="ExternalInput").ap(),
        "wp": nc.dram_tensor("wp", [F, C], F32R, kind="ExternalInput").ap(),
        "cmask": nc.dram_tensor("cmask", [128, 1024], F32,
                                kind="ExternalInput").ap(),
        "out_pa": nc.dram_tensor("out_pa", [T, C], F32,
                                 kind="ExternalOutput").ap(),
        "out_pb": nc.dram_tensor("out_pb", [T, C], F32,
                                 kind="ExternalOutput").ap(),
    }
    with tile.TileContext(nc) as tc:
        _emit(tc, aps)
    nc.compile()
    _CACHE["nc"] = nc
    return nc


def _make_in_maps(x, Wqkv, bqkv, Wproj):
    x = np.asarray(x, dtype=np.float32)
    Wqkv = np.asarray(Wqkv, dtype=np.float32)
    bqkv = np.asarray(bqkv, dtype=np.float32)
    Wproj = np.asarray(Wproj, dtype=np.float32)

    # sliding causal mask: M[p, u] = 0 if u >= p + 384 else -1e9
    p_idx = np.arange(128)[:, None]
    u_idx = np.arange(1024)[None, :]
    cmask = np.where(u_idx >= p_idx + 384, 0.0, -1e9).astype(np.float32)

    in_maps = []
    for core in range(N_CORES):
        b, g = divmod(core, 2)
        q0, k0, v0 = 512 * g, C + 512 * g, 2 * C + 512 * g
        wva = np.zeros((C, VW), dtype=np.float32)
        bva = np.zeros((1, VW), dtype=np.float32)
        for h in range(NH):
            src = v0 + D * h
            dst = 65 * h
            # per-head layout [v(64), one]
            wva[:, dst:dst + 64] = Wqkv[:, src:src + 64]
            bva[0, dst:dst + 64] = bqkv[src:src + 64]
            bva[0, dst + 64] = 1.0
        in_maps.append({
            "x": np.ascontiguousarray(x[b]),
            "wq": np.ascontiguousarray(Wqkv[:, q0:q0 + F]),
            "wk": np.ascontiguousarray(Wqkv[:, k0:k0 + F]),
            "wva": wva,
            "bq": np.ascontiguousarray(bqkv[q0:q0 + F].reshape(F, 1)),
            "bk": np.ascontiguousarray(bqkv[k0:k0 + F].reshape(F, 1)),
            "bva": bva,
            "wp": np.ascontiguousarray(Wproj[512 * g:512 * g + F, :]),
            "cmask": cmask,
        })
    return in_maps


def run_sharded(x, Wqkv, bqkv, Wproj, bproj, trace=False):
    nc = _build()
    in_maps = _make_in_maps(x, Wqkv, bqkv, Wproj)
    res = run_bass_kernel_spmd(nc, in_maps, core_ids=list(range(N_CORES)),
                               trace=trace)
    bproj = np.asarray(bproj, dtype=np.float32)
    out = np.empty((B, T, C), dtype=np.float32)
    for b in range(B):
        acc = bproj[None, :].astype(np.float32).repeat(T, axis=0)
        for core in (2 * b, 2 * b + 1):
            acc = acc + res.results[core]["out_pa"] + res.results[core]["out_pb"]
        out[b] = acc
    return out, res


def kernel(x, Wqkv, bqkv, Wproj, bproj):
    out, _ = run_sharded(x, Wqkv, bqkv, Wproj, bproj, trace=False)
    return out


# revision 22
# speedup vs baseline: 1.1686x; 1.1686x over previous
"""Causal self-attention (B=4, T=2048, C=1024, H=16, Dh=64) on 8 trn2 NeuronCores.

Sharding: core i <-> (batch b = i//2, head-group g = i%2). Each core computes
8 heads of one batch end-to-end (qkv slice, causal attention, partial output
projection); the host sums the head-group/pair-couple partials per batch and
adds bproj. No device collectives.

All matmuls run as float32r (single-pass reduced-precision fp32 on the PE,
full-rate at moving-dim >= 256), accumulating in fp32 PSUM. Attention uses
the transposed-scores layout sT[tk, tq] so no per-block transposes are
needed: softmax denominators come out of the PV matmul via an extra ones
column interleaved into Wv, and are broadcast across partitions with a
partition-step-0 SBUF->SBUF DMA.
"""

import numpy as np

import concourse.bass as bass
import concourse.tile as tile
from concourse import bacc, mybir
from concourse.bass_utils import run_bass_kernel_spmd
from concourse.masks import make_identity

F32 = mybir.dt.float32
F32R = mybir.dt.float32r

N_CORES = 8
B, T, C = 4, 2048, 1024
NH_TOT, D = 16, 64
F = 512            # features per core (8 heads)
NH = 8             # local heads
NPAIR = 4          # head pairs (128 feats each)
CCH = C // 128     # 8 contraction chunks
NTT = T // 128     # 16 t tiles
NTB = T // 512     # 4 t blocks (qkv production)
NQB = T // 512     # 4 q blocks (attention)
VW = NH * (D + 1)  # 520: augmented v width
ADD = mybir.AluOpType.add
MULT = mybir.AluOpType.mult


def _emit(tc, aps):
    from contextlib import ExitStack
    nc = tc.nc
    x, wq, wk, wva, bq, bk, bva, wp = (
        aps["x"], aps["wq"], aps["wk"], aps["wva"], aps["bq"], aps["bk"],
        aps["bva"], aps["wp"])
    cmask = aps["cmask"]
    out_ab = [aps["out_pa"], aps["out_pb"]]

    # ---- pools (all coexist; ~210KB/partition total) ----
    ctx = ExitStack()
    pp_qk = ctx.enter_context(tc.tile_pool(name="ps_qk", bufs=2, space="PSUM"))
    pp_s = ctx.enter_context(tc.tile_pool(name="ps_s", bufs=2, space="PSUM"))
    pp_pv = ctx.enter_context(tc.tile_pool(name="ps_pv", bufs=2, space="PSUM"))
    po_v = ctx.enter_context(tc.tile_pool(name="v_all", bufs=1))
    po_mask = ctx.enter_context(tc.tile_pool(name="mask", bufs=1))
    po_wv = ctx.enter_context(tc.tile_pool(name="wv", bufs=8))
    po_qkt = ctx.enter_context(tc.tile_pool(name="qkT", bufs=2))
    po_bias = ctx.enter_context(tc.tile_pool(name="bias", bufs=1))
    po_misc = ctx.enter_context(tc.tile_pool(name="misc", bufs=2))
    po_xt = ctx.enter_context(tc.tile_pool(name="xT", bufs=1))
    po_wqk = ctx.enter_context(tc.tile_pool(name="wqk", bufs=8))
    po_yt = ctx.enter_context(tc.tile_pool(name="yT", bufs=2))
    po_exp = ctx.enter_context(tc.tile_pool(name="expT", bufs=3))
    po_rec = ctx.enter_context(tc.tile_pool(name="recip", bufs=2))
    po_den = ctx.enter_context(tc.tile_pool(name="den", bufs=1))
    po_ytmp = ctx.enter_context(tc.tile_pool(name="ytmp", bufs=1))
    po_wp = ctx.enter_context(tc.tile_pool(name="wp", bufs=4))

    mask_sb = po_mask.tile([128, 512], F32, tag="mask")
    nc.sync.dma_start(out=mask_sb[:], in_=cmask[:])
    bva_sb = po_bias.tile([1, VW], F32R, tag="bva")
    nc.sync.dma_start(out=bva_sb[:], in_=bva[:])
    ones_f32 = po_bias.tile([128, 128], F32, tag="ones_f32")
    nc.vector.memset(ones_f32[:], 1.0)
    ones_row = po_bias.tile([1, 128], F32R, tag="ones")
    nc.vector.tensor_copy(ones_row[:], ones_f32[0:1, :])
    ones64 = po_bias.tile([128, 64], F32R, tag="ones64")
    nc.vector.tensor_copy(ones64[:], ones_f32[:, 0:64])
    ident = po_bias.tile([128, 128], F32, tag="ident")
    make_identity(nc, ident[:])

    # ---- phase 0: x -> xT via PE transposes ----
    xT = [po_xt.tile([128, T], F32R, tag=f"xT{c}", name=f"xT{c}")
          for c in range(CCH)]
    for tt in range(NTT):
        xt_in = po_misc.tile([128, C], F32, tag="misc", name="xt_in")
        nc.sync.dma_start(out=xt_in[:], in_=x[tt * 128:(tt + 1) * 128, :])
        tsl = slice(tt * 128, (tt + 1) * 128)
        for ca in range(2):
            pst = pp_qk.tile([128, 512], F32, tag="qk")
            for j in range(4):
                c = ca * 4 + j
                nc.tensor.transpose(
                    pst[:, j * 128:(j + 1) * 128],
                    xt_in[:, c * 128:(c + 1) * 128],
                    ident[:])
            for j in range(4):
                nc.vector.tensor_copy(
                    xT[ca * 4 + j][:, tsl], pst[:, j * 128:(j + 1) * 128])

    # ---- phase 0b: v (augmented with ones columns, all 8 heads) ----
    v_all = [po_v.tile([128, VW], F32R, tag=f"v{tt}", name=f"v{tt}")
             for tt in range(NTT)]
    for half in range(2):
        cs = slice(half * 260, half * 260 + 260)
        wv_sb = []
        for c in range(CCH):
            wt = po_wv.tile([128, 260], F32R, tag="wv")
            nc.sync.dma_start(out=wt[:], in_=wva[c * 128:(c + 1) * 128, cs])
            wv_sb.append(wt)
        for tt in range(NTT):
            ps = pp_qk.tile([128, 260], F32, tag="qk")
            for c in range(CCH):
                nc.tensor.matmul(
                    ps[:], xT[c][:, tt * 128:(tt + 1) * 128],
                    wv_sb[c][:], start=(c == 0), stop=False)
            nc.tensor.matmul(ps[:], ones_row[:], bva_sb[:, cs],
                             start=False, stop=True)
            nc.vector.tensor_copy(v_all[tt][:, cs], ps[:])

    # ---- per head pair: qkv -> attention -> partial proj ----
    yt_couple = []
    for pair in range(NPAIR):
        psl = slice(pair * 128, (pair + 1) * 128)

        # qT / kT for this pair
        wqk_c = []
        for c in range(CCH):
            wt = po_wqk.tile([128, 256], F32R, tag="wqk")
            nc.sync.dma_start(out=wt[:, 0:128],
                              in_=wq[c * 128:(c + 1) * 128, psl])
            nc.sync.dma_start(out=wt[:, 128:256],
                              in_=wk[c * 128:(c + 1) * 128, psl])
            wqk_c.append(wt)
        bq_sb = po_bias.tile([128, 1], F32, tag=f"bq{pair}", name=f"bq{pair}")
        nc.sync.dma_start(out=bq_sb[:], in_=bq[psl, :])
        bk_sb = po_bias.tile([128, 1], F32, tag=f"bk{pair}", name=f"bk{pair}")
        nc.sync.dma_start(out=bk_sb[:], in_=bk[psl, :])

        qT = po_qkt.tile([128, T], F32R, tag="qT")
        kT = po_qkt.tile([128, T], F32R, tag="kT")
        for tb in range(NTB):
            tsl = slice(tb * 512, (tb + 1) * 512)
            psq = pp_qk.tile([128, 512], F32, tag="qk")
            for c in range(CCH):
                nc.tensor.matmul(psq[:], wqk_c[c][:, 0:128],
                                 xT[c][:, tsl],
                                 start=(c == 0), stop=(c == CCH - 1))
            # (x@wq)*1/sqrt(D) + bq/sqrt(D)   (bq pre-scaled on host)
            nc.scalar.activation(
                qT[:, tsl], psq[:], mybir.ActivationFunctionType.Identity,
                bias=bq_sb[:], scale=0.125)
            psk = pp_qk.tile([128, 512], F32, tag="qk")
            for c in range(CCH):
                nc.tensor.matmul(psk[:], wqk_c[c][:, 128:256],
                                 xT[c][:, tsl],
                                 start=(c == 0), stop=(c == CCH - 1))
            nc.scalar.activation(
                kT[:, tsl], psk[:], mybir.ActivationFunctionType.Identity,
                bias=bk_sb[:], scale=1.0)

        # attention for the pair's two heads
        yt = po_yt.tile([128, T], F32R, tag="yT")
        for hl in range(2):
            h = pair * 2 + hl
            rq = slice(hl * 64, hl * 64 + 64)
            vsl = slice(h * 65, h * 65 + 65)
            for qb in range(NQB):
                qsl = slice(qb * 512, (qb + 1) * 512)
                nkt = 4 * qb + 4
                pv = pp_pv.tile([128, 512], F32, tag="pv")
                for g0 in range(0, nkt, 2):
                    gs = min(2, nkt - g0)
                    st = pp_s.tile([128, 1024], F32, tag="s")
                    offs = []
                    for i in range(gs):
                        kt = g0 + i
                        j = kt - 4 * qb
                        # columns tq < 128*j of a diagonal tile are entirely
                        # masked -> skip them in scores, mask-add and PV
                        off = 128 * j if j > 0 else 0
                        offs.append(off)
                        nc.tensor.matmul(
                            st[:, i * 512 + off:(i + 1) * 512],
                            kT[rq, kt * 128:(kt + 1) * 128],
                            qT[rq, qb * 512 + off:(qb + 1) * 512],
                            start=True, stop=True)
                        if j >= 0:  # triangular mask on the remaining block
                            nc.vector.tensor_add(
                                st[:, i * 512 + off:(i + 1) * 512],
                                st[:, i * 512 + off:(i + 1) * 512],
                                mask_sb[:, 0:512 - off])
                    et = po_exp.tile([128, 1024], F32R, tag="expT")
                    if gs == 2 and offs == [0, 0]:
                        nc.scalar.activation(
                            et[:, 0:1024], st[:, 0:1024],
                            mybir.ActivationFunctionType.Exp)
                    else:
                        for i in range(gs):
                            off = offs[i]
                            nc.scalar.activation(
                                et[:, i * 512 + off:(i + 1) * 512],
                                st[:, i * 512 + off:(i + 1) * 512],
                                mybir.ActivationFunctionType.Exp)
                    for i in range(gs):
                        kt = g0 + i
                        off = offs[i]
                        nc.tensor.matmul(
                            pv[0:65, off:512], v_all[kt][:, vsl],
                            et[:, i * 512 + off:(i + 1) * 512],
                            start=(kt == 0), stop=(kt == nkt - 1))
                # normalization: den row 64 -> broadcast -> recip -> mul
                # den row 64 -> (PE outer with ones) broadcast -> recip
                den = po_den.tile([128, 512], F32R, tag="den")
                nc.vector.tensor_copy(den[64:65, :], pv[64:65, :])
                bcp = pp_qk.tile([128, 512], F32, tag="qk")
                nc.tensor.matmul(bcp[0:64, :], ones64[64:65, :],
                                 den[64:65, :], start=True, stop=True)
                rec = po_rec.tile([128, 512], F32, tag="recip")
                nc.vector.reciprocal_approx_fast(rec[0:64, :], bcp[0:64, :])
                if hl == 0:
                    nc.vector.tensor_mul(yt[0:64, qsl], pv[0:64, :],
                                         rec[0:64, :])
                else:
                    # y must land on partitions 64..127 of the pair tile;
                    # engines can't cross partitions, so bounce via DMA.
                    ytmp = po_ytmp.tile([128, 512], F32R, tag="ytmp")
                    nc.vector.tensor_mul(ytmp[0:64, :], pv[0:64, :],
                                         rec[0:64, :])
                    nc.sync.dma_start(out=yt[64:128, qsl],
                                      in_=ytmp[0:64, :])
        yt_couple.append(yt)

        # partial projection per pair-couple (pairs 0+1 -> out_pa, 2+3 -> out_pb)
        if pair % 2 == 1:
            out_p = out_ab[pair // 2]
            wp_sb = []
            for pq in range(2):
                for cb in range(2):
                    prow = (pair - 1 + pq) * 128
                    wt = po_wp.tile([128, 512], F32R, tag="wp")
                    nc.sync.dma_start(
                        out=wt[:],
                        in_=wp[prow:prow + 128, cb * 512:(cb + 1) * 512])
                    wp_sb.append(wt)
            for tt in range(NTT):
                ot = po_misc.tile([128, C], F32, tag="misc", name="ot")
                for cb in range(2):
                    ps = pp_qk.tile([128, 512], F32, tag="qk")
                    for pq in range(2):
                        nc.tensor.matmul(
                            ps[:],
                            yt_couple[pq][:, tt * 128:(tt + 1) * 128],
                            wp_sb[pq * 2 + cb][:],
                            start=(pq == 0), stop=(pq == 1))
                    nc.vector.tensor_copy(ot[:, cb * 512:(cb + 1) * 512], ps[:])
                nc.sync.dma_start(out=out_p[tt * 128:(tt + 1) * 128, :],
                                  in_=ot[:])
            yt_couple = []
    ctx.close()


_CACHE = {}


def _build():
    if "nc" in _CACHE:
        return _CACHE["nc"]
    nc = bacc.Bacc("TRN2", target_bir_lowering=False, debug=False,
                   enable_asserts=True, num_devices=N_CORES)
    aps = {
        "x": nc.dram_tensor("x", [T, C], F32, kind="ExternalInput").ap(),
        "wq": nc.dram_tensor("wq", [C, F], F32R, kind="ExternalInput").ap(),
        "wk": nc.dram_tensor("wk", [C, F], F32R, kind="ExternalInput").ap(),
        "wva": nc.dram_tensor("wva", [C, VW], F32R, kind="ExternalInput").ap(),
        "bq": nc.dram_tensor("bq", [F, 1], F32, kind="ExternalInput").ap(),
        "bk": nc.dram_tensor("bk", [F, 1], F32, kind="ExternalInput").ap(),
        "bva": nc.dram_tensor("bva", [1, VW], F32R, kind="ExternalInput").ap(),
        "wp": nc.dram_tensor("wp", [F, C], F32R, kind="ExternalInput").ap(),
        "cmask": nc.dram_tensor("cmask", [128, 512], F32,
                                kind="ExternalInput").ap(),
        "out_pa": nc.dram_tensor("out_pa", [T, C], F32,
                                 kind="ExternalOutput").ap(),
        "out_pb": nc.dram_tensor("out_pb", [T, C], F32,
                                 kind="ExternalOutput").ap(),
    }
    with tile.TileContext(nc) as tc:
        _emit(tc, aps)
    nc.compile()
    _CACHE["nc"] = nc
    return nc


def _make_in_maps(x, Wqkv, bqkv, Wproj):
    x = np.asarray(x, dtype=np.float32)
    Wqkv = np.asarray(Wqkv, dtype=np.float32)
    bqkv = np.asarray(bqkv, dtype=np.float32)
    Wproj = np.asarray(Wproj, dtype=np.float32)

    # triangular causal mask: M[p, f] = 0 if f >= p else -1e9
    p_idx = np.arange(128)[:, None]
    u_idx = np.arange(512)[None, :]
    cmask = np.where(u_idx >= p_idx, 0.0, -1e9).astype(np.float32)

    in_maps = []
    for core in range(N_CORES):
        b, g = divmod(core, 2)
        q0, k0, v0 = 512 * g, C + 512 * g, 2 * C + 512 * g
        wva = np.zeros((C, VW), dtype=np.float32)
        bva = np.zeros((1, VW), dtype=np.float32)
        for h in range(NH):
            src = v0 + D * h
            dst = 65 * h
            # per-head layout [v(64), one]
            wva[:, dst:dst + 64] = Wqkv[:, src:src + 64]
            bva[0, dst:dst + 64] = bqkv[src:src + 64]
            bva[0, dst + 64] = 1.0
        in_maps.append({
            "x": np.ascontiguousarray(x[b]),
            "wq": np.ascontiguousarray(Wqkv[:, q0:q0 + F]),
            "wk": np.ascontiguousarray(Wqkv[:, k0:k0 + F]),
            "wva": wva,
            "bq": np.ascontiguousarray(bqkv[q0:q0 + F].reshape(F, 1) * 0.125),
            "bk": np.ascontiguousarray(bqkv[k0:k0 + F].reshape(F, 1)),
            "bva": bva,
            "wp": np.ascontiguousarray(Wproj[512 * g:512 * g + F, :]),
            "cmask": cmask,
        })
    return in_maps


def run_sharded(x, Wqkv, bqkv, Wproj, bproj, trace=False):
    nc = _build()
    in_maps = _make_in_maps(x, Wqkv, bqkv, Wproj)
    res = run_bass_kernel_spmd(nc, in_maps, core_ids=list(range(N_CORES)),
                               trace=trace)
    bproj = np.asarray(bproj, dtype=np.float32)
    out = np.empty((B, T, C), dtype=np.float32)
    for b in range(B):
        acc = bproj[None, :].astype(np.float32).repeat(T, axis=0)
        for core in (2 * b, 2 * b + 1):
            acc = acc + res.results[core]["out_pa"] + res.results[core]["out_pb"]
        out[b] = acc
    return out, res


def kernel(x, Wqkv, bqkv, Wproj, bproj):
    out, _ = run_sharded(x, Wqkv, bqkv, Wproj, bproj, trace=False)
    return out


# revision 23
# speedup vs baseline: 1.3117x; 1.1224x over previous
"""Causal self-attention (B=4, T=2048, C=1024, H=16, Dh=64) on 8 trn2 NeuronCores.

Sharding: core i <-> (batch b = i//2, head-group g = i%2). Each core computes
8 heads of one batch end-to-end (qkv slice, causal attention, partial output
projection); the host sums the head-group/pair-couple partials per batch and
adds bproj. No device collectives.

All matmuls run as float32r (single-pass reduced-precision fp32 on the PE,
full-rate at moving-dim >= 256), accumulating in fp32 PSUM. Attention uses
the transposed-scores layout sT[tk, tq] so no per-block transposes are
needed: softmax denominators come out of the PV matmul via an extra ones
column interleaved into Wv, and are broadcast across partitions with a
partition-step-0 SBUF->SBUF DMA.
"""

import numpy as np

import concourse.bass as bass
import concourse.tile as tile
from concourse import bacc, mybir
from concourse.bass_utils import run_bass_kernel_spmd
from concourse.masks import make_identity

F32 = mybir.dt.float32
F32R = mybir.dt.float32r

N_CORES = 8
B, T, C = 4, 2048, 1024
NH_TOT, D = 16, 64
F = 512            # features per core (8 heads)
NH = 8             # local heads
NPAIR = 4          # head pairs (128 feats each)
CCH = C // 128     # 8 contraction chunks
NTT = T // 128     # 16 t tiles
NTB = T // 512     # 4 t blocks (qkv production)
NQB = T // 512     # 4 q blocks (attention)
VW = NH * (D + 1)  # 520: augmented v width
ADD = mybir.AluOpType.add
MULT = mybir.AluOpType.mult


def _emit(tc, aps):
    from contextlib import ExitStack
    nc = tc.nc
    x, wq, wk, wva, bq, bk, bva, wp = (
        aps["x"], aps["wq"], aps["wk"], aps["wva"], aps["bq"], aps["bk"],
        aps["bva"], aps["wp"])
    cmask = aps["cmask"]
    out_ab = [aps["out_pa"], aps["out_pb"]]

    # ---- pools (all coexist; ~210KB/partition total) ----
    ctx = ExitStack()
    pp_qk = ctx.enter_context(tc.tile_pool(name="ps_qk", bufs=2, space="PSUM"))
    pp_s = ctx.enter_context(tc.tile_pool(name="ps_s", bufs=2, space="PSUM"))
    pp_pv = ctx.enter_context(tc.tile_pool(name="ps_pv", bufs=2, space="PSUM"))
    po_v = ctx.enter_context(tc.tile_pool(name="v_all", bufs=1))
    po_mask = ctx.enter_context(tc.tile_pool(name="mask", bufs=1))
    po_wv = ctx.enter_context(tc.tile_pool(name="wv", bufs=8))
    po_qkt = ctx.enter_context(tc.tile_pool(name="qkT", bufs=2))
    po_bias = ctx.enter_context(tc.tile_pool(name="bias", bufs=1))
    po_misc = ctx.enter_context(tc.tile_pool(name="misc", bufs=2))
    po_xt = ctx.enter_context(tc.tile_pool(name="xT", bufs=1))
    po_wqk = ctx.enter_context(tc.tile_pool(name="wqk", bufs=8))
    po_yt = ctx.enter_context(tc.tile_pool(name="yT", bufs=2))
    po_exp = ctx.enter_context(tc.tile_pool(name="expT", bufs=3))
    po_rec = ctx.enter_context(tc.tile_pool(name="recip", bufs=2))
    po_den = ctx.enter_context(tc.tile_pool(name="den", bufs=1))
    po_ytmp = ctx.enter_context(tc.tile_pool(name="ytmp", bufs=1))
    po_wp = ctx.enter_context(tc.tile_pool(name="wp", bufs=4))
    po_dram = ctx.enter_context(tc.tile_pool(name="dram_scr", bufs=4,
                                             space="DRAM"))

    mask_sb = po_mask.tile([128, 512], F32, tag="mask")
    nc.sync.dma_start(out=mask_sb[:], in_=cmask[:])
    ident = po_bias.tile([128, 128], F32, tag="ident")
    make_identity(nc, ident[:])
    # bva broadcast to all 128 partitions straight from DRAM
    bva_bc = po_bias.tile([128, VW], F32, tag="bva_bc")
    bva2 = aps["bva2"]
    nc.sync.dma_start(out=bva_bc[:], in_=bass.AP(
        tensor=bva2.tensor, offset=bva2.offset,
        ap=[[0, 128]] + [list(a) for a in bva2.ap[1:]]))

    # ---- phase 0: x -> xT via PE transposes ----
    xT = [po_xt.tile([128, T], F32R, tag=f"xT{c}", name=f"xT{c}")
          for c in range(CCH)]
    for tt in range(NTT):
        xt_in = po_misc.tile([128, C], F32, tag="misc", name="xt_in")
        nc.sync.dma_start(out=xt_in[:], in_=x[tt * 128:(tt + 1) * 128, :])
        tsl = slice(tt * 128, (tt + 1) * 128)
        for ca in range(2):
            pst = pp_qk.tile([128, 512], F32, tag="qk")
            for j in range(4):
                c = ca * 4 + j
                nc.tensor.transpose(
                    pst[:, j * 128:(j + 1) * 128],
                    xt_in[:, c * 128:(c + 1) * 128],
                    ident[:])
            for j in range(4):
                nc.vector.tensor_copy(
                    xT[ca * 4 + j][:, tsl], pst[:, j * 128:(j + 1) * 128])

    # ---- phase 0b: v (augmented with ones columns, all 8 heads) ----
    v_all = [po_v.tile([128, VW], F32R, tag=f"v{tt}", name=f"v{tt}")
             for tt in range(NTT)]
    for half in range(2):
        cs = slice(half * 260, half * 260 + 260)
        wv_sb = []
        for c in range(CCH):
            wt = po_wv.tile([128, 260], F32R, tag="wv")
            nc.sync.dma_start(out=wt[:], in_=wva[c * 128:(c + 1) * 128, cs])
            wv_sb.append(wt)
        for tt in range(NTT):
            ps = pp_qk.tile([128, 260], F32, tag="qk")
            for c in range(CCH):
                nc.tensor.matmul(
                    ps[:], xT[c][:, tt * 128:(tt + 1) * 128],
                    wv_sb[c][:], start=(c == 0), stop=(c == CCH - 1))
            nc.vector.tensor_add(v_all[tt][:, cs], ps[:], bva_bc[:, cs])

    # ---- per head pair: qkv -> attention -> partial proj ----
    yt_couple = []
    for pair in range(NPAIR):
        psl = slice(pair * 128, (pair + 1) * 128)

        # qT / kT for this pair
        wqk_c = []
        for c in range(CCH):
            wt = po_wqk.tile([128, 256], F32R, tag="wqk")
            nc.sync.dma_start(out=wt[:, 0:128],
                              in_=wq[c * 128:(c + 1) * 128, psl])
            nc.sync.dma_start(out=wt[:, 128:256],
                              in_=wk[c * 128:(c + 1) * 128, psl])
            wqk_c.append(wt)
        bq_sb = po_bias.tile([128, 1], F32, tag=f"bq{pair}", name=f"bq{pair}")
        nc.sync.dma_start(out=bq_sb[:], in_=bq[psl, :])
        bk_sb = po_bias.tile([128, 1], F32, tag=f"bk{pair}", name=f"bk{pair}")
        nc.sync.dma_start(out=bk_sb[:], in_=bk[psl, :])

        qT = po_qkt.tile([128, T], F32R, tag="qT")
        kT = po_qkt.tile([128, T], F32R, tag="kT")
        for tb in range(NTB):
            tsl = slice(tb * 512, (tb + 1) * 512)
            psq = pp_qk.tile([128, 512], F32, tag="qk")
            for c in range(CCH):
                nc.tensor.matmul(psq[:], wqk_c[c][:, 0:128],
                                 xT[c][:, tsl],
                                 start=(c == 0), stop=(c == CCH - 1))
            # (x@wq)*1/sqrt(D) + bq/sqrt(D)   (bq pre-scaled on host)
            nc.scalar.activation(
                qT[:, tsl], psq[:], mybir.ActivationFunctionType.Identity,
                bias=bq_sb[:], scale=0.125)
            psk = pp_qk.tile([128, 512], F32, tag="qk")
            for c in range(CCH):
                nc.tensor.matmul(psk[:], wqk_c[c][:, 128:256],
                                 xT[c][:, tsl],
                                 start=(c == 0), stop=(c == CCH - 1))
            nc.scalar.activation(
                kT[:, tsl], psk[:], mybir.ActivationFunctionType.Identity,
                bias=bk_sb[:], scale=1.0)

        # attention for the pair's two heads
        yt = po_yt.tile([128, T], F32R, tag="yT")
        for hl in range(2):
            h = pair * 2 + hl
            rq = slice(hl * 64, hl * 64 + 64)
            vsl = slice(h * 65, h * 65 + 65)
            for qb in range(NQB):
                qsl = slice(qb * 512, (qb + 1) * 512)
                nkt = 4 * qb + 4
                pv = pp_pv.tile([128, 512], F32, tag="pv")
                def emit_scores(g0, gs, st, offs):
                    for i in range(gs):
                        kt = g0 + i
                        j = kt - 4 * qb
                        # columns tq < 128*j of a diagonal tile are entirely
                        # masked -> skip them in scores, mask-add and PV
                        off = 128 * j if j > 0 else 0
                        offs.append(off)
                        nc.tensor.matmul(
                            st[:, i * 512 + off:(i + 1) * 512],
                            kT[rq, kt * 128:(kt + 1) * 128],
                            qT[rq, qb * 512 + off:(qb + 1) * 512],
                            start=True, stop=True)
                        if j >= 0:  # triangular mask on the remaining block
                            nc.vector.tensor_add(
                                st[:, i * 512 + off:(i + 1) * 512],
                                st[:, i * 512 + off:(i + 1) * 512],
                                mask_sb[:, 0:512 - off])

                def emit_exp_pv(g0, gs, st, offs):
                    et = po_exp.tile([128, 1024], F32R, tag="expT",
                                     name="et")
                    if gs == 2 and offs == [0, 0]:
                        nc.scalar.activation(
                            et[:, 0:1024], st[:, 0:1024],
                            mybir.ActivationFunctionType.Exp)
                    else:
                        for i in range(gs):
                            off = offs[i]
                            nc.scalar.activation(
                                et[:, i * 512 + off:(i + 1) * 512],
                                st[:, i * 512 + off:(i + 1) * 512],
                                mybir.ActivationFunctionType.Exp)
                    for i in range(gs):
                        kt = g0 + i
                        off = offs[i]
                        nc.tensor.matmul(
                            pv[0:65, off:512], v_all[kt][:, vsl],
                            et[:, i * 512 + off:(i + 1) * 512],
                            start=(kt == 0), stop=(kt == nkt - 1))

                pend = None
                for g0 in range(0, nkt, 2):
                    gs = min(2, nkt - g0)
                    st = pp_s.tile([128, 1024], F32, tag="s")
                    offs = []
                    emit_scores(g0, gs, st, offs)
                    if pend is not None:
                        emit_exp_pv(*pend)
                    pend = (g0, gs, st, offs)
                emit_exp_pv(*pend)
                # den row 64 -> DRAM bounce -> partition broadcast -> recip
                den = po_den.tile([128, 512], F32, tag="den")
                nc.vector.tensor_copy(den[64:65, :], pv[64:65, :])
                dscr = po_dram.tile([1, 512], F32, tag="dscr", name="dscr")
                nc.sync.dma_start(out=dscr[:], in_=den[64:65, :])
                rec = po_rec.tile([128, 512], F32, tag="recip")
                nc.sync.dma_start(out=rec[0:64, :], in_=bass.AP(
                    tensor=dscr.tensor, offset=dscr[:].offset,
                    ap=[[0, 64]] + [list(a) for a in dscr[:].ap[1:]]))
                nc.vector.reciprocal_approx_fast(rec[0:64, :], rec[0:64, :])
                if hl == 0:
                    nc.vector.tensor_mul(yt[0:64, qsl], pv[0:64, :],
                                         rec[0:64, :])
                else:
                    # y must land on partitions 64..127 of the pair tile;
                    # engines can't cross partitions, so bounce via DMA.
                    ytmp = po_ytmp.tile([128, 512], F32R, tag="ytmp")
                    nc.vector.tensor_mul(ytmp[0:64, :], pv[0:64, :],
                                         rec[0:64, :])
                    nc.sync.dma_start(out=yt[64:128, qsl],
                                      in_=ytmp[0:64, :])
        yt_couple.append(yt)

        # partial projection per pair-couple (pairs 0+1 -> out_pa, 2+3 -> out_pb)
        if pair % 2 == 1:
            out_p = out_ab[pair // 2]
            wp_sb = []
            for pq in range(2):
                for cb in range(2):
                    prow = (pair - 1 + pq) * 128
                    wt = po_wp.tile([128, 512], F32R, tag="wp")
                    nc.sync.dma_start(
                        out=wt[:],
                        in_=wp[prow:prow + 128, cb * 512:(cb + 1) * 512])
                    wp_sb.append(wt)
            for tt in range(NTT):
                ot = po_misc.tile([128, C], F32, tag="misc", name="ot")
                for cb in range(2):
                    ps = pp_qk.tile([128, 512], F32, tag="qk")
                    for pq in range(2):
                        nc.tensor.matmul(
                            ps[:],
                            yt_couple[pq][:, tt * 128:(tt + 1) * 128],
                            wp_sb[pq * 2 + cb][:],
                            start=(pq == 0), stop=(pq == 1))
                    nc.vector.tensor_copy(ot[:, cb * 512:(cb + 1) * 512], ps[:])
                nc.sync.dma_start(out=out_p[tt * 128:(tt + 1) * 128, :],
                                  in_=ot[:])
            yt_couple = []
    ctx.close()


_CACHE = {}


def _build():
    if "nc" in _CACHE:
        return _CACHE["nc"]
    nc = bacc.Bacc("TRN2", target_bir_lowering=False, debug=False,
                   enable_asserts=True, num_devices=N_CORES)
    aps = {
        "x": nc.dram_tensor("x", [T, C], F32, kind="ExternalInput").ap(),
        "wq": nc.dram_tensor("wq", [C, F], F32R, kind="ExternalInput").ap(),
        "wk": nc.dram_tensor("wk", [C, F], F32R, kind="ExternalInput").ap(),
        "wva": nc.dram_tensor("wva", [C, VW], F32R, kind="ExternalInput").ap(),
        "bq": nc.dram_tensor("bq", [F, 1], F32, kind="ExternalInput").ap(),
        "bk": nc.dram_tensor("bk", [F, 1], F32, kind="ExternalInput").ap(),
        "bva": nc.dram_tensor("bva", [1, VW], F32R, kind="ExternalInput").ap(),
        "bva2": nc.dram_tensor("bva2", [1, VW], F32, kind="ExternalInput").ap(),
        "wp": nc.dram_tensor("wp", [F, C], F32R, kind="ExternalInput").ap(),
        "cmask": nc.dram_tensor("cmask", [128, 512], F32,
                                kind="ExternalInput").ap(),
        "out_pa": nc.dram_tensor("out_pa", [T, C], F32,
                                 kind="ExternalOutput").ap(),
        "out_pb": nc.dram_tensor("out_pb", [T, C], F32,
                                 kind="ExternalOutput").ap(),
    }
    with tile.TileContext(nc) as tc:
        _emit(tc, aps)
    nc.compile()
    _CACHE["nc"] = nc
    return nc


def _make_in_maps(x, Wqkv, bqkv, Wproj):
    x = np.asarray(x, dtype=np.float32)
    Wqkv = np.asarray(Wqkv, dtype=np.float32)
    bqkv = np.asarray(bqkv, dtype=np.float32)
    Wproj = np.asarray(Wproj, dtype=np.float32)

    # triangular causal mask: M[p, f] = 0 if f >= p else -1e9
    p_idx = np.arange(128)[:, None]
    u_idx = np.arange(512)[None, :]
    cmask = np.where(u_idx >= p_idx, 0.0, -1e9).astype(np.float32)

    in_maps = []
    for core in range(N_CORES):
        b, g = divmod(core, 2)
        q0, k0, v0 = 512 * g, C + 512 * g, 2 * C + 512 * g
        wva = np.zeros((C, VW), dtype=np.float32)
        bva = np.zeros((1, VW), dtype=np.float32)
        for h in range(NH):
            src = v0 + D * h
            dst = 65 * h
            # per-head layout [v(64), one]
            wva[:, dst:dst + 64] = Wqkv[:, src:src + 64]
            bva[0, dst:dst + 64] = bqkv[src:src + 64]
            bva[0, dst + 64] = 1.0
        in_maps.append({
            "x": np.ascontiguousarray(x[b]),
            "wq": np.ascontiguousarray(Wqkv[:, q0:q0 + F]),
            "wk": np.ascontiguousarray(Wqkv[:, k0:k0 + F]),
            "wva": wva,
            "bq": np.ascontiguousarray(bqkv[q0:q0 + F].reshape(F, 1) * 0.125),
            "bk": np.ascontiguousarray(bqkv[k0:k0 + F].reshape(F, 1)),
            "bva": bva,
            "bva2": bva,
            "wp": np.ascontiguousarray(Wproj[512 * g:512 * g + F, :]),
            "cmask": cmask,
        })
    return in_maps


def run_sharded(x, Wqkv, bqkv, Wproj, bproj, trace=False):
    nc = _build()
    in_maps = _make_in_maps(x, Wqkv, bqkv, Wproj)
    res = run_bass_kernel_spmd(nc, in_maps, core_ids=list(range(N_CORES)),
                               trace=trace)
    bproj = np.asarray(bproj, dtype=np.float32)
    out = np.empty((B, T, C), dtype=np.float32)
    for b in range(B):
        acc = bproj[None, :].astype(np.float32).repeat(T, axis=0)
        for core in (2 * b, 2 * b + 1):
            acc = acc + res.results[core]["out_pa"] + res.results[core]["out_pb"]
        out[b] = acc
    return out, res


def kernel(x, Wqkv, bqkv, Wproj, bproj):
    out, _ = run_sharded(x, Wqkv, bqkv, Wproj, bproj, trace=False)
    return out


# revision 25
# speedup vs baseline: 1.5472x; 1.1795x over previous
"""Causal self-attention (B=4, T=2048, C=1024, H=16, Dh=64) on 8 trn2 NeuronCores.

Sharding: core i <-> (batch b = i//2, head-group g = i%2). Each core computes
8 heads of one batch end-to-end (qkv slice, causal attention, partial output
projection); the host sums the head-group/pair-couple partials per batch and
adds bproj. No device collectives.

All matmuls run as float32r (single-pass reduced-precision fp32 on the PE,
full-rate at moving-dim >= 256), accumulating in fp32 PSUM. Attention uses
the transposed-scores layout sT[tk, tq] so no per-block transposes are
needed: softmax denominators come out of the PV matmul via an extra ones
column interleaved into Wv, and are broadcast across partitions with a
partition-step-0 SBUF->SBUF DMA.
"""

import numpy as np

import concourse.bass as bass
import concourse.tile as tile
from concourse import bacc, mybir
from concourse.bass_utils import run_bass_kernel_spmd
from concourse.masks import make_identity

F32 = mybir.dt.float32
F32R = mybir.dt.float32r
F16 = mybir.dt.float16

N_CORES = 8
B, T, C = 4, 2048, 1024
NH_TOT, D = 16, 64
F = 512            # features per core (8 heads)
NH = 8             # local heads
NPAIR = 4          # head pairs (128 feats each)
CCH = C // 128     # 8 contraction chunks
NTT = T // 128     # 16 t tiles
NTB = T // 512     # 4 t blocks (qkv production)
NQB = T // 512     # 4 q blocks (attention)
VW = NH * (D + 1)  # 520: augmented v width
ADD = mybir.AluOpType.add
MULT = mybir.AluOpType.mult


def _emit(tc, aps):
    from contextlib import ExitStack
    nc = tc.nc
    x, wq, wk, wva, bq, bk, wp = (
        aps["x"], aps["wq"], aps["wk"], aps["wva"], aps["bq"], aps["bk"],
        aps["wp"])
    cmask = aps["cmask"]
    out_ab = [aps["out_pa"], aps["out_pb"]]

    # ---- pools (all coexist; ~210KB/partition total) ----
    ctx = ExitStack()
    pp_qk = ctx.enter_context(tc.tile_pool(name="ps_qk", bufs=2, space="PSUM"))
    pp_s = ctx.enter_context(tc.tile_pool(name="ps_s", bufs=2, space="PSUM"))
    pp_pv = ctx.enter_context(tc.tile_pool(name="ps_pv", bufs=2, space="PSUM"))
    po_v = ctx.enter_context(tc.tile_pool(name="v_all", bufs=1))
    po_mask = ctx.enter_context(tc.tile_pool(name="mask", bufs=1))
    po_wv = ctx.enter_context(tc.tile_pool(name="wv", bufs=8))
    po_qkt = ctx.enter_context(tc.tile_pool(name="qkT", bufs=2))
    po_bias = ctx.enter_context(tc.tile_pool(name="bias", bufs=1))
    po_misc = ctx.enter_context(tc.tile_pool(name="misc", bufs=2))
    po_xt = ctx.enter_context(tc.tile_pool(name="xT", bufs=1))
    po_wqk = ctx.enter_context(tc.tile_pool(name="wqk", bufs=8))
    po_yt = ctx.enter_context(tc.tile_pool(name="yT", bufs=2))
    po_exp = ctx.enter_context(tc.tile_pool(name="expT", bufs=3))
    po_rec = ctx.enter_context(tc.tile_pool(name="recip", bufs=2))
    po_den = ctx.enter_context(tc.tile_pool(name="den", bufs=1))
    po_ytmp = ctx.enter_context(tc.tile_pool(name="ytmp", bufs=1))
    po_wp = ctx.enter_context(tc.tile_pool(name="wp", bufs=4))
    po_dram = ctx.enter_context(tc.tile_pool(name="dram_scr", bufs=4,
                                             space="DRAM"))

    mask_sb = po_mask.tile([128, 512], F32, tag="mask")
    nc.sync.dma_start(out=mask_sb[:], in_=cmask[:])
    ident = po_bias.tile([128, 128], F16, tag="ident")
    make_identity(nc, ident[:])
    # bva broadcast to all 128 partitions straight from DRAM
    bva_bc = po_bias.tile([128, VW], F32, tag="bva_bc")
    bva2 = aps["bva2"]
    nc.sync.dma_start(out=bva_bc[:], in_=bass.AP(
        tensor=bva2.tensor, offset=bva2.offset,
        ap=[[0, 128]] + [list(a) for a in bva2.ap[1:]]))

    # ---- phase 0: x -> xT via PE transposes ----
    xT = [po_xt.tile([128, T], F16, tag=f"xT{c}", name=f"xT{c}")
          for c in range(CCH)]
    for tt in range(NTT):
        xt_in = po_misc.tile([128, C], F16, tag="miscH", name="xt_in")
        nc.sync.dma_start(out=xt_in[:], in_=x[tt * 128:(tt + 1) * 128, :])
        tsl = slice(tt * 128, (tt + 1) * 128)
        for ca in range(2):
            pst = pp_qk.tile([128, 512], F16, tag="qk", name="pst")
            for j in range(4):
                c = ca * 4 + j
                nc.tensor.transpose(
                    pst[:, j * 128:(j + 1) * 128],
                    xt_in[:, c * 128:(c + 1) * 128],
                    ident[:])
            for j in range(4):
                nc.vector.tensor_copy(
                    xT[ca * 4 + j][:, tsl], pst[:, j * 128:(j + 1) * 128])

    # ---- phase 0b: v (augmented with ones columns, all 8 heads) ----
    v_all = [po_v.tile([128, VW], F16, tag=f"v{tt}", name=f"v{tt}")
             for tt in range(NTT)]
    for half in range(2):
        cs = slice(half * 260, half * 260 + 260)
        wv_sb = []
        for c in range(CCH):
            wt = po_wv.tile([128, 260], F16, tag="wv")
            nc.sync.dma_start(out=wt[:], in_=wva[c * 128:(c + 1) * 128, cs])
            wv_sb.append(wt)
        for tt in range(NTT):
            ps = pp_qk.tile([128, 260], F32, tag="qk")
            for c in range(CCH):
                nc.tensor.matmul(
                    ps[:], xT[c][:, tt * 128:(tt + 1) * 128],
                    wv_sb[c][:], start=(c == 0), stop=(c == CCH - 1))
            nc.vector.tensor_add(v_all[tt][:, cs], ps[:], bva_bc[:, cs])

    # ---- per head pair: qkv -> attention -> partial proj ----
    yt_couple = []
    for pair in range(NPAIR):
        psl = slice(pair * 128, (pair + 1) * 128)

        # qT / kT for this pair
        wqk_c = []
        for c in range(CCH):
            wt = po_wqk.tile([128, 256], F16, tag="wqk")
            nc.sync.dma_start(out=wt[:, 0:128],
                              in_=wq[c * 128:(c + 1) * 128, psl])
            nc.sync.dma_start(out=wt[:, 128:256],
                              in_=wk[c * 128:(c + 1) * 128, psl])
            wqk_c.append(wt)
        bq_sb = po_bias.tile([128, 1], F32, tag=f"bq{pair}", name=f"bq{pair}")
        nc.sync.dma_start(out=bq_sb[:], in_=bq[psl, :])
        bk_sb = po_bias.tile([128, 1], F32, tag=f"bk{pair}", name=f"bk{pair}")
        nc.sync.dma_start(out=bk_sb[:], in_=bk[psl, :])

        qT = po_qkt.tile([128, T], F16, tag="qT")
        kT = po_qkt.tile([128, T], F16, tag="kT")
        for tb in range(NTB):
            tsl = slice(tb * 512, (tb + 1) * 512)
            psq = pp_qk.tile([128, 512], F32, tag="qk")
            for c in range(CCH):
                nc.tensor.matmul(psq[:], wqk_c[c][:, 0:128],
                                 xT[c][:, tsl],
                                 start=(c == 0), stop=(c == CCH - 1))
            # (x@wq)*1/sqrt(D) + bq/sqrt(D)   (bq pre-scaled on host)
            nc.scalar.activation(
                qT[:, tsl], psq[:], mybir.ActivationFunctionType.Identity,
                bias=bq_sb[:], scale=0.125)
            psk = pp_qk.tile([128, 512], F32, tag="qk")
            for c in range(CCH):
                nc.tensor.matmul(psk[:], wqk_c[c][:, 128:256],
                                 xT[c][:, tsl],
                                 start=(c == 0), stop=(c == CCH - 1))
            nc.scalar.activation(
                kT[:, tsl], psk[:], mybir.ActivationFunctionType.Identity,
                bias=bk_sb[:], scale=1.0)

        # attention for the pair's two heads
        yt = po_yt.tile([128, T], F16, tag="yT")
        for hl in range(2):
            h = pair * 2 + hl
            rq = slice(hl * 64, hl * 64 + 64)
            vsl = slice(h * 65, h * 65 + 65)
            for qb in range(NQB):
                qsl = slice(qb * 512, (qb + 1) * 512)
                nkt = 4 * qb + 4
                pv = pp_pv.tile([128, 512], F32, tag="pv")
                def emit_scores(g0, gs, st, offs):
                    for i in range(gs):
                        kt = g0 + i
                        j = kt - 4 * qb
                        # columns tq < 128*j of a diagonal tile are entirely
                        # masked -> skip them in scores, mask-add and PV
                        off = 128 * j if j > 0 else 0
                        offs.append(off)
                        nc.tensor.matmul(
                            st[:, i * 512 + off:(i + 1) * 512],
                            kT[rq, kt * 128:(kt + 1) * 128],
                            qT[rq, qb * 512 + off:(qb + 1) * 512],
                            start=True, stop=True)
                        if j >= 0:  # triangular mask on the remaining block
                            nc.vector.tensor_add(
                                st[:, i * 512 + off:(i + 1) * 512],
                                st[:, i * 512 + off:(i + 1) * 512],
                                mask_sb[:, 0:512 - off])

                def emit_exp_pv(g0, gs, st, offs):
                    et = po_exp.tile([128, 1024], F16, tag="expT",
                                     name="et")
                    if gs == 2 and offs == [0, 0]:
                        nc.scalar.activation(
                            et[:, 0:1024], st[:, 0:1024],
                            mybir.ActivationFunctionType.Exp)
                    else:
                        for i in range(gs):
                            off = offs[i]
                            nc.scalar.activation(
                                et[:, i * 512 + off:(i + 1) * 512],
                                st[:, i * 512 + off:(i + 1) * 512],
                                mybir.ActivationFunctionType.Exp)
                    for i in range(gs):
                        kt = g0 + i
                        off = offs[i]
                        nc.tensor.matmul(
                            pv[0:65, off:512], v_all[kt][:, vsl],
                            et[:, i * 512 + off:(i + 1) * 512],
                            start=(kt == 0), stop=(kt == nkt - 1))

                pend = None
                for g0 in range(0, nkt, 2):
                    gs = min(2, nkt - g0)
                    st = pp_s.tile([128, 1024], F32, tag="s")
                    offs = []
                    emit_scores(g0, gs, st, offs)
                    if pend is not None:
                        emit_exp_pv(*pend)
                    pend = (g0, gs, st, offs)
                emit_exp_pv(*pend)
                # den row 64 -> DRAM bounce -> partition broadcast -> recip
                den = po_den.tile([128, 512], F32, tag="den")
                nc.vector.tensor_copy(den[64:65, :], pv[64:65, :])
                dscr = po_dram.tile([1, 512], F32, tag="dscr", name="dscr")
                nc.sync.dma_start(out=dscr[:], in_=den[64:65, :])
                rec = po_rec.tile([128, 512], F32, tag="recip")
                nc.sync.dma_start(out=rec[0:64, :], in_=bass.AP(
                    tensor=dscr.tensor, offset=dscr[:].offset,
                    ap=[[0, 64]] + [list(a) for a in dscr[:].ap[1:]]))
                nc.vector.reciprocal_approx_fast(rec[0:64, :], rec[0:64, :])
                if hl == 0:
                    nc.vector.tensor_mul(yt[0:64, qsl], pv[0:64, :],
                                         rec[0:64, :])
                else:
                    # y must land on partitions 64..127 of the pair tile;
                    # engines can't cross partitions, so bounce via DMA.
                    ytmp = po_ytmp.tile([128, 512], F16, tag="ytmp")
                    nc.vector.tensor_mul(ytmp[0:64, :], pv[0:64, :],
                                         rec[0:64, :])
                    nc.sync.dma_start(out=yt[64:128, qsl],
                                      in_=ytmp[0:64, :])
        yt_couple.append(yt)

        # partial projection per pair-couple (pairs 0+1 -> out_pa, 2+3 -> out_pb)
        if pair % 2 == 1:
            out_p = out_ab[pair // 2]
            wp_sb = []
            for pq in range(2):
                for cb in range(2):
                    prow = (pair - 1 + pq) * 128
                    wt = po_wp.tile([128, 512], F16, tag="wp")
                    nc.sync.dma_start(
                        out=wt[:],
                        in_=wp[prow:prow + 128, cb * 512:(cb + 1) * 512])
                    wp_sb.append(wt)
            for tt in range(NTT):
                ot = po_misc.tile([128, C], F32, tag="misc", name="ot")
                for cb in range(2):
                    ps = pp_qk.tile([128, 512], F32, tag="qk")
                    for pq in range(2):
                        nc.tensor.matmul(
                            ps[:],
                            yt_couple[pq][:, tt * 128:(tt + 1) * 128],
                            wp_sb[pq * 2 + cb][:],
                            start=(pq == 0), stop=(pq == 1))
                    nc.vector.tensor_copy(ot[:, cb * 512:(cb + 1) * 512], ps[:])
                nc.sync.dma_start(out=out_p[tt * 128:(tt + 1) * 128, :],
                                  in_=ot[:])
            yt_couple = []
    ctx.close()


_CACHE = {}


def _build():
    if "nc" in _CACHE:
        return _CACHE["nc"]
    nc = bacc.Bacc("TRN2", target_bir_lowering=False, debug=False,
                   enable_asserts=True, num_devices=N_CORES)
    aps = {
        "x": nc.dram_tensor("x", [T, C], F16, kind="ExternalInput").ap(),
        "wq": nc.dram_tensor("wq", [C, F], F16, kind="ExternalInput").ap(),
        "wk": nc.dram_tensor("wk", [C, F], F16, kind="ExternalInput").ap(),
        "wva": nc.dram_tensor("wva", [C, VW], F16, kind="ExternalInput").ap(),
        "bq": nc.dram_tensor("bq", [F, 1], F32, kind="ExternalInput").ap(),
        "bk": nc.dram_tensor("bk", [F, 1], F32, kind="ExternalInput").ap(),
        "bva2": nc.dram_tensor("bva2", [1, VW], F32, kind="ExternalInput").ap(),
        "wp": nc.dram_tensor("wp", [F, C], F16, kind="ExternalInput").ap(),
        "cmask": nc.dram_tensor("cmask", [128, 512], F32,
                                kind="ExternalInput").ap(),
        "out_pa": nc.dram_tensor("out_pa", [T, C], F32,
                                 kind="ExternalOutput").ap(),
        "out_pb": nc.dram_tensor("out_pb", [T, C], F32,
                                 kind="ExternalOutput").ap(),
    }
    with tile.TileContext(nc) as tc:
        _emit(tc, aps)
    nc.compile()
    _CACHE["nc"] = nc
    return nc


def _make_in_maps(x, Wqkv, bqkv, Wproj):
    x = np.asarray(x, dtype=np.float32)
    Wqkv = np.asarray(Wqkv, dtype=np.float32)
    bqkv = np.asarray(bqkv, dtype=np.float32)
    Wproj = np.asarray(Wproj, dtype=np.float32)

    # triangular causal mask: M[p, f] = 0 if f >= p else -1e9
    p_idx = np.arange(128)[:, None]
    u_idx = np.arange(512)[None, :]
    cmask = np.where(u_idx >= p_idx, 0.0, -1e9).astype(np.float32)

    in_maps = []
    for core in range(N_CORES):
        b, g = divmod(core, 2)
        q0, k0, v0 = 512 * g, C + 512 * g, 2 * C + 512 * g
        wva = np.zeros((C, VW), dtype=np.float32)
        bva = np.zeros((1, VW), dtype=np.float32)
        for h in range(NH):
            src = v0 + D * h
            dst = 65 * h
            # per-head layout [v(64), one]
            wva[:, dst:dst + 64] = Wqkv[:, src:src + 64]
            bva[0, dst:dst + 64] = bqkv[src:src + 64]
            bva[0, dst + 64] = 1.0
        in_maps.append({
            "x": np.ascontiguousarray(x[b]).astype(np.float16),
            "wq": np.ascontiguousarray(Wqkv[:, q0:q0 + F]).astype(np.float16),
            "wk": np.ascontiguousarray(Wqkv[:, k0:k0 + F]).astype(np.float16),
            "wva": wva.astype(np.float16),
            "bq": np.ascontiguousarray(bqkv[q0:q0 + F].reshape(F, 1) * 0.125),
            "bk": np.ascontiguousarray(bqkv[k0:k0 + F].reshape(F, 1)),
            "bva2": bva,
            "wp": np.ascontiguousarray(Wproj[512 * g:512 * g + F, :]).astype(np.float16),
            "cmask": cmask,
        })
    return in_maps


def run_sharded(x, Wqkv, bqkv, Wproj, bproj, trace=False):
    nc = _build()
    in_maps = _make_in_maps(x, Wqkv, bqkv, Wproj)
    res = run_bass_kernel_spmd(nc, in_maps, core_ids=list(range(N_CORES)),
                               trace=trace)
    bproj = np.asarray(bproj, dtype=np.float32)
    out = np.empty((B, T, C), dtype=np.float32)
    for b in range(B):
        acc = bproj[None, :].astype(np.float32).repeat(T, axis=0)
        for core in (2 * b, 2 * b + 1):
            acc = acc + res.results[core]["out_pa"] + res.results[core]["out_pb"]
        out[b] = acc
    return out, res


def kernel(x, Wqkv, bqkv, Wproj, bproj):
    out, _ = run_sharded(x, Wqkv, bqkv, Wproj, bproj, trace=False)
    return out


# revision 26
# speedup vs baseline: 1.6087x; 1.0398x over previous
"""Causal self-attention (B=4, T=2048, C=1024, H=16, Dh=64) on 8 trn2 NeuronCores.

Sharding: core i <-> (batch b = i//2, head-group g = i%2). Each core computes
8 heads of one batch end-to-end (qkv slice, causal attention, partial output
projection); the host sums the head-group/pair-couple partials per batch and
adds bproj. No device collectives.

All matmuls run as float32r (single-pass reduced-precision fp32 on the PE,
full-rate at moving-dim >= 256), accumulating in fp32 PSUM. Attention uses
the transposed-scores layout sT[tk, tq] so no per-block transposes are
needed: softmax denominators come out of the PV matmul via an extra ones
column interleaved into Wv, and are broadcast across partitions with a
partition-step-0 SBUF->SBUF DMA.
"""

import numpy as np

import concourse.bass as bass
import concourse.tile as tile
from concourse import bacc, mybir
from concourse.bass_utils import run_bass_kernel_spmd
from concourse.masks import make_identity

F32 = mybir.dt.float32
F32R = mybir.dt.float32r
F16 = mybir.dt.float16

N_CORES = 8
B, T, C = 4, 2048, 1024
NH_TOT, D = 16, 64
F = 512            # features per core (8 heads)
NH = 8             # local heads
NPAIR = 4          # head pairs (128 feats each)
CCH = C // 128     # 8 contraction chunks
NTT = T // 128     # 16 t tiles
NTB = T // 512     # 4 t blocks (qkv production)
NQB = T // 512     # 4 q blocks (attention)
VW = NH * (D + 1)  # 520: augmented v width
ADD = mybir.AluOpType.add
MULT = mybir.AluOpType.mult


def _emit(tc, aps):
    from contextlib import ExitStack
    nc = tc.nc
    x, wq, wk, wva, bq, bk, wp = (
        aps["x"], aps["wq"], aps["wk"], aps["wva"], aps["bq"], aps["bk"],
        aps["wp"])
    cmask = aps["cmask"]
    out_ab = [aps["out_pa"], aps["out_pb"]]

    # ---- pools (all coexist; ~210KB/partition total) ----
    ctx = ExitStack()
    pp_qk = ctx.enter_context(tc.tile_pool(name="ps_qk", bufs=2, space="PSUM"))
    pp_s = ctx.enter_context(tc.tile_pool(name="ps_s", bufs=2, space="PSUM"))
    pp_pv = ctx.enter_context(tc.tile_pool(name="ps_pv", bufs=2, space="PSUM"))
    po_v = ctx.enter_context(tc.tile_pool(name="v_all", bufs=1))
    po_mask = ctx.enter_context(tc.tile_pool(name="mask", bufs=1))
    po_wv = ctx.enter_context(tc.tile_pool(name="wv", bufs=8))
    po_qkt = ctx.enter_context(tc.tile_pool(name="qkT", bufs=2))
    po_bias = ctx.enter_context(tc.tile_pool(name="bias", bufs=1))
    po_misc = ctx.enter_context(tc.tile_pool(name="misc", bufs=3))
    po_xt = ctx.enter_context(tc.tile_pool(name="xT", bufs=1))
    po_wqk = ctx.enter_context(tc.tile_pool(name="wqk", bufs=8))
    po_yt = ctx.enter_context(tc.tile_pool(name="yT", bufs=2))
    po_exp = ctx.enter_context(tc.tile_pool(name="expT", bufs=4))
    po_rec = ctx.enter_context(tc.tile_pool(name="recip", bufs=3))
    po_den = ctx.enter_context(tc.tile_pool(name="den", bufs=2))
    po_ytmp = ctx.enter_context(tc.tile_pool(name="ytmp", bufs=2))
    po_wp = ctx.enter_context(tc.tile_pool(name="wp", bufs=4))
    po_dram = ctx.enter_context(tc.tile_pool(name="dram_scr", bufs=4,
                                             space="DRAM"))

    mask_sb = po_mask.tile([128, 512], F32, tag="mask")
    nc.sync.dma_start(out=mask_sb[:], in_=cmask[:])
    ident = po_bias.tile([128, 128], F16, tag="ident")
    nc.sync.dma_start(out=ident[:], in_=aps["identin"][:])
    # bva broadcast to all 128 partitions straight from DRAM
    bva_bc = po_bias.tile([128, VW], F32, tag="bva_bc")
    bva2 = aps["bva2"]
    nc.sync.dma_start(out=bva_bc[:], in_=bass.AP(
        tensor=bva2.tensor, offset=bva2.offset,
        ap=[[0, 128]] + [list(a) for a in bva2.ap[1:]]))

    # ---- phase 0: x -> xT via PE transposes ----
    xT = [po_xt.tile([128, T], F16, tag=f"xT{c}", name=f"xT{c}")
          for c in range(CCH)]
    for tt in range(NTT):
        xt_in = po_misc.tile([128, C], F16, tag="miscH", name="xt_in")
        nc.sync.dma_start(out=xt_in[:], in_=x[tt * 128:(tt + 1) * 128, :])
        tsl = slice(tt * 128, (tt + 1) * 128)
        for ca in range(2):
            pst = pp_qk.tile([128, 512], F16, tag="qk", name="pst")
            for j in range(4):
                c = ca * 4 + j
                nc.tensor.transpose(
                    pst[:, j * 128:(j + 1) * 128],
                    xt_in[:, c * 128:(c + 1) * 128],
                    ident[:])
            for j in range(4):
                nc.vector.tensor_copy(
                    xT[ca * 4 + j][:, tsl], pst[:, j * 128:(j + 1) * 128])

    # ---- phase 0b: v (augmented with ones columns, all 8 heads) ----
    v_all = [po_v.tile([128, VW], F16, tag=f"v{tt}", name=f"v{tt}")
             for tt in range(NTT)]
    for half in range(2):
        cs = slice(half * 260, half * 260 + 260)
        wv_sb = []
        for c in range(CCH):
            wt = po_wv.tile([128, 260], F16, tag="wv")
            nc.sync.dma_start(out=wt[:], in_=wva[c * 128:(c + 1) * 128, cs])
            wv_sb.append(wt)
        for tt in range(NTT):
            ps = pp_qk.tile([128, 260], F32, tag="qk")
            for c in range(CCH):
                nc.tensor.matmul(
                    ps[:], xT[c][:, tt * 128:(tt + 1) * 128],
                    wv_sb[c][:], start=(c == 0), stop=(c == CCH - 1))
            nc.vector.tensor_add(v_all[tt][:, cs], ps[:], bva_bc[:, cs])

    # ---- per head pair: qkv -> attention -> partial proj ----
    yt_couple = []
    for pair in range(NPAIR):
        psl = slice(pair * 128, (pair + 1) * 128)

        # qT / kT for this pair
        wqk_c = []
        for c in range(CCH):
            wt = po_wqk.tile([128, 256], F16, tag="wqk")
            nc.sync.dma_start(out=wt[:, 0:128],
                              in_=wq[c * 128:(c + 1) * 128, psl])
            nc.sync.dma_start(out=wt[:, 128:256],
                              in_=wk[c * 128:(c + 1) * 128, psl])
            wqk_c.append(wt)
        bq_sb = po_bias.tile([128, 1], F32, tag=f"bq{pair}", name=f"bq{pair}")
        nc.sync.dma_start(out=bq_sb[:], in_=bq[psl, :])
        bk_sb = po_bias.tile([128, 1], F32, tag=f"bk{pair}", name=f"bk{pair}")
        nc.sync.dma_start(out=bk_sb[:], in_=bk[psl, :])

        qT = po_qkt.tile([128, T], F16, tag="qT")
        kT = po_qkt.tile([128, T], F16, tag="kT")
        for tb in range(NTB):
            tsl = slice(tb * 512, (tb + 1) * 512)
            psq = pp_qk.tile([128, 512], F32, tag="qk")
            for c in range(CCH):
                nc.tensor.matmul(psq[:], wqk_c[c][:, 0:128],
                                 xT[c][:, tsl],
                                 start=(c == 0), stop=(c == CCH - 1))
            # psum*1/sqrt(D) + bq/sqrt(D)   (bq pre-scaled on host)
            nc.vector.tensor_scalar(
                out=qT[:, tsl], in0=psq[:], scalar1=0.125,
                scalar2=bq_sb[:], op0=MULT, op1=ADD)
            psk = pp_qk.tile([128, 512], F32, tag="qk")
            for c in range(CCH):
                nc.tensor.matmul(psk[:], wqk_c[c][:, 128:256],
                                 xT[c][:, tsl],
                                 start=(c == 0), stop=(c == CCH - 1))
            nc.vector.tensor_scalar(
                out=kT[:, tsl], in0=psk[:], scalar1=bk_sb[:],
                scalar2=None, op0=ADD)

        # attention for the pair's two heads
        yt = po_yt.tile([128, T], F16, tag="yT")
        for hl in range(2):
            h = pair * 2 + hl
            rq = slice(hl * 64, hl * 64 + 64)
            vsl = slice(h * 65, h * 65 + 65)
            for qb in range(NQB):
                qsl = slice(qb * 512, (qb + 1) * 512)
                nkt = 4 * qb + 4
                pv = pp_pv.tile([128, 512], F32, tag="pv")
                def emit_scores(g0, gs, st, offs):
                    for i in range(gs):
                        kt = g0 + i
                        j = kt - 4 * qb
                        # columns tq < 128*j of a diagonal tile are entirely
                        # masked -> skip them in scores, mask-add and PV
                        off = 128 * j if j > 0 else 0
                        offs.append(off)
                        nc.tensor.matmul(
                            st[:, i * 512 + off:(i + 1) * 512],
                            kT[rq, kt * 128:(kt + 1) * 128],
                            qT[rq, qb * 512 + off:(qb + 1) * 512],
                            start=True, stop=True)
                        if j >= 0:  # triangular mask on the remaining block
                            nc.vector.tensor_add(
                                st[:, i * 512 + off:(i + 1) * 512],
                                st[:, i * 512 + off:(i + 1) * 512],
                                mask_sb[:, 0:512 - off])

                def emit_exp_pv(g0, gs, st, offs):
                    et = po_exp.tile([128, 1024], F16, tag="expT",
                                     name="et")
                    if gs == 2 and offs == [0, 0]:
                        nc.scalar.activation(
                            et[:, 0:1024], st[:, 0:1024],
                            mybir.ActivationFunctionType.Exp)
                    else:
                        for i in range(gs):
                            off = offs[i]
                            nc.scalar.activation(
                                et[:, i * 512 + off:(i + 1) * 512],
                                st[:, i * 512 + off:(i + 1) * 512],
                                mybir.ActivationFunctionType.Exp)
                    for i in range(gs):
                        kt = g0 + i
                        off = offs[i]
                        nc.tensor.matmul(
                            pv[0:65, off:512], v_all[kt][:, vsl],
                            et[:, i * 512 + off:(i + 1) * 512],
                            start=(kt == 0), stop=(kt == nkt - 1))

                pend = None
                for g0 in range(0, nkt, 2):
                    gs = min(2, nkt - g0)
                    st = pp_s.tile([128, 1024], F32, tag="s")
                    offs = []
                    emit_scores(g0, gs, st, offs)
                    if pend is not None:
                        emit_exp_pv(*pend)
                    pend = (g0, gs, st, offs)
                emit_exp_pv(*pend)
                # den row 64 -> DRAM bounce -> partition broadcast -> recip
                den = po_den.tile([128, 512], F32, tag="den")
                nc.vector.tensor_copy(den[64:65, :], pv[64:65, :])
                dscr = po_dram.tile([1, 512], F32, tag="dscr", name="dscr")
                nc.sync.dma_start(out=dscr[:], in_=den[64:65, :])
                rec = po_rec.tile([128, 512], F32, tag="recip")
                nc.sync.dma_start(out=rec[0:64, :], in_=bass.AP(
                    tensor=dscr.tensor, offset=dscr[:].offset,
                    ap=[[0, 64]] + [list(a) for a in dscr[:].ap[1:]]))
                nc.vector.reciprocal_approx_fast(rec[0:64, :], rec[0:64, :])
                if hl == 0:
                    nc.vector.tensor_mul(yt[0:64, qsl], pv[0:64, :],
                                         rec[0:64, :])
                else:
                    # y must land on partitions 64..127 of the pair tile;
                    # engines can't cross partitions, so bounce via DMA.
                    ytmp = po_ytmp.tile([128, 512], F16, tag="ytmp")
                    nc.vector.tensor_mul(ytmp[0:64, :], pv[0:64, :],
                                         rec[0:64, :])
                    nc.sync.dma_start(out=yt[64:128, qsl],
                                      in_=ytmp[0:64, :])
        yt_couple.append(yt)

        # partial projection per pair-couple (pairs 0+1 -> out_pa, 2+3 -> out_pb)
        if pair % 2 == 1:
            out_p = out_ab[pair // 2]
            wp_sb = []
            for pq in range(2):
                for cb in range(2):
                    prow = (pair - 1 + pq) * 128
                    wt = po_wp.tile([128, 512], F16, tag="wp")
                    nc.sync.dma_start(
                        out=wt[:],
                        in_=wp[prow:prow + 128, cb * 512:(cb + 1) * 512])
                    wp_sb.append(wt)
            for tt in range(NTT):
                ot = po_misc.tile([128, C], F32, tag="misc", name="ot")
                for cb in range(2):
                    ps = pp_qk.tile([128, 512], F32, tag="qk")
                    for pq in range(2):
                        nc.tensor.matmul(
                            ps[:],
                            yt_couple[pq][:, tt * 128:(tt + 1) * 128],
                            wp_sb[pq * 2 + cb][:],
                            start=(pq == 0), stop=(pq == 1))
                    nc.vector.tensor_copy(ot[:, cb * 512:(cb + 1) * 512], ps[:])
                nc.sync.dma_start(out=out_p[tt * 128:(tt + 1) * 128, :],
                                  in_=ot[:])
            yt_couple = []
    ctx.close()


_CACHE = {}


def _build():
    if "nc" in _CACHE:
        return _CACHE["nc"]
    nc = bacc.Bacc("TRN2", target_bir_lowering=False, debug=False,
                   enable_asserts=True, num_devices=N_CORES)
    aps = {
        "x": nc.dram_tensor("x", [T, C], F16, kind="ExternalInput").ap(),
        "wq": nc.dram_tensor("wq", [C, F], F16, kind="ExternalInput").ap(),
        "wk": nc.dram_tensor("wk", [C, F], F16, kind="ExternalInput").ap(),
        "wva": nc.dram_tensor("wva", [C, VW], F16, kind="ExternalInput").ap(),
        "bq": nc.dram_tensor("bq", [F, 1], F32, kind="ExternalInput").ap(),
        "bk": nc.dram_tensor("bk", [F, 1], F32, kind="ExternalInput").ap(),
        "bva2": nc.dram_tensor("bva2", [1, VW], F32, kind="ExternalInput").ap(),
        "wp": nc.dram_tensor("wp", [F, C], F16, kind="ExternalInput").ap(),
        "cmask": nc.dram_tensor("cmask", [128, 512], F32,
                                kind="ExternalInput").ap(),
        "identin": nc.dram_tensor("identin", [128, 128], F16,
                                  kind="ExternalInput").ap(),
        "out_pa": nc.dram_tensor("out_pa", [T, C], F32,
                                 kind="ExternalOutput").ap(),
        "out_pb": nc.dram_tensor("out_pb", [T, C], F32,
                                 kind="ExternalOutput").ap(),
    }
    with tile.TileContext(nc) as tc:
        _emit(tc, aps)
    nc.compile()
    _CACHE["nc"] = nc
    return nc


def _make_in_maps(x, Wqkv, bqkv, Wproj):
    x = np.asarray(x, dtype=np.float32)
    Wqkv = np.asarray(Wqkv, dtype=np.float32)
    bqkv = np.asarray(bqkv, dtype=np.float32)
    Wproj = np.asarray(Wproj, dtype=np.float32)

    # triangular causal mask: M[p, f] = 0 if f >= p else -1e9
    p_idx = np.arange(128)[:, None]
    u_idx = np.arange(512)[None, :]
    cmask = np.where(u_idx >= p_idx, 0.0, -1e9).astype(np.float32)

    in_maps = []
    for core in range(N_CORES):
        b, g = divmod(core, 2)
        q0, k0, v0 = 512 * g, C + 512 * g, 2 * C + 512 * g
        wva = np.zeros((C, VW), dtype=np.float32)
        bva = np.zeros((1, VW), dtype=np.float32)
        for h in range(NH):
            src = v0 + D * h
            dst = 65 * h
            # per-head layout [v(64), one]
            wva[:, dst:dst + 64] = Wqkv[:, src:src + 64]
            bva[0, dst:dst + 64] = bqkv[src:src + 64]
            bva[0, dst + 64] = 1.0
        in_maps.append({
            "x": np.ascontiguousarray(x[b]).astype(np.float16),
            "wq": np.ascontiguousarray(Wqkv[:, q0:q0 + F]).astype(np.float16),
            "wk": np.ascontiguousarray(Wqkv[:, k0:k0 + F]).astype(np.float16),
            "wva": wva.astype(np.float16),
            "bq": np.ascontiguousarray(bqkv[q0:q0 + F].reshape(F, 1) * 0.125),
            "bk": np.ascontiguousarray(bqkv[k0:k0 + F].reshape(F, 1)),
            "bva2": bva,
            "wp": np.ascontiguousarray(Wproj[512 * g:512 * g + F, :]).astype(np.float16),
            "cmask": cmask,
            "identin": np.eye(128, dtype=np.float16),
        })
    return in_maps


def run_sharded(x, Wqkv, bqkv, Wproj, bproj, trace=False):
    nc = _build()
    in_maps = _make_in_maps(x, Wqkv, bqkv, Wproj)
    res = run_bass_kernel_spmd(nc, in_maps, core_ids=list(range(N_CORES)),
                               trace=trace)
    bproj = np.asarray(bproj, dtype=np.float32)
    out = np.empty((B, T, C), dtype=np.float32)
    for b in range(B):
        acc = bproj[None, :].astype(np.float32).repeat(T, axis=0)
        for core in (2 * b, 2 * b + 1):
            acc = acc + res.results[core]["out_pa"] + res.results[core]["out_pb"]
        out[b] = acc
    return out, res


def kernel(x, Wqkv, bqkv, Wproj, bproj):
    out, _ = run_sharded(x, Wqkv, bqkv, Wproj, bproj, trace=False)
    return out


# revision 28
# speedup vs baseline: 1.6817x; 1.0454x over previous
"""Causal self-attention (B=4, T=2048, C=1024, H=16, Dh=64) on 8 trn2 NeuronCores.

Sharding: core i <-> (batch b = i//2, head-group g = i%2). Each core computes
8 heads of one batch end-to-end (qkv slice, causal attention, partial output
projection); the host sums the head-group/pair-couple partials per batch and
adds bproj. No device collectives.

All matmuls run as float32r (single-pass reduced-precision fp32 on the PE,
full-rate at moving-dim >= 256), accumulating in fp32 PSUM. Attention uses
the transposed-scores layout sT[tk, tq] so no per-block transposes are
needed: softmax denominators come out of the PV matmul via an extra ones
column interleaved into Wv, and are broadcast across partitions with a
partition-step-0 SBUF->SBUF DMA.
"""

import numpy as np

import concourse.bass as bass
import concourse.tile as tile
from concourse import bacc, mybir
from concourse.bass_utils import run_bass_kernel_spmd
from concourse.masks import make_identity

F32 = mybir.dt.float32
F32R = mybir.dt.float32r
F16 = mybir.dt.float16

N_CORES = 8
B, T, C = 4, 2048, 1024
NH_TOT, D = 16, 64
F = 512            # features per core (8 heads)
NH = 8             # local heads
NPAIR = 4          # head pairs (128 feats each)
CCH = C // 128     # 8 contraction chunks
NTT = T // 128     # 16 t tiles
NTB = T // 512     # 4 t blocks (qkv production)
NQB = T // 512     # 4 q blocks (attention)
VW = NH * (D + 1)  # 520: augmented v width
ADD = mybir.AluOpType.add
MULT = mybir.AluOpType.mult


def _emit(tc, aps):
    from contextlib import ExitStack
    nc = tc.nc
    x, wq, wk, wva, bq, bk, wp = (
        aps["x"], aps["wq"], aps["wk"], aps["wva"], aps["bq"], aps["bk"],
        aps["wp"])
    cmask = aps["cmask"]
    out_ab = [aps["out_pa"], aps["out_pb"]]

    # ---- pools (all coexist; ~210KB/partition total) ----
    ctx = ExitStack()
    pp_qk = ctx.enter_context(tc.tile_pool(name="ps_qk", bufs=2, space="PSUM"))
    pp_s = ctx.enter_context(tc.tile_pool(name="ps_s", bufs=2, space="PSUM"))
    pp_pv = ctx.enter_context(tc.tile_pool(name="ps_pv", bufs=2, space="PSUM"))
    po_v = ctx.enter_context(tc.tile_pool(name="v_all", bufs=1))
    po_mask = ctx.enter_context(tc.tile_pool(name="mask", bufs=1))
    po_wv = ctx.enter_context(tc.tile_pool(name="wv", bufs=8))
    po_qkt = ctx.enter_context(tc.tile_pool(name="qkT", bufs=2))
    po_bias = ctx.enter_context(tc.tile_pool(name="bias", bufs=1))
    po_misc = ctx.enter_context(tc.tile_pool(name="misc", bufs=3))
    po_xt = ctx.enter_context(tc.tile_pool(name="xT", bufs=1))
    po_wqk = ctx.enter_context(tc.tile_pool(name="wqk", bufs=8))
    po_yt = ctx.enter_context(tc.tile_pool(name="yT", bufs=3))
    po_exp = ctx.enter_context(tc.tile_pool(name="expT", bufs=4))
    po_rec = ctx.enter_context(tc.tile_pool(name="recip", bufs=3))
    po_den = ctx.enter_context(tc.tile_pool(name="den", bufs=2))
    po_ytmp = ctx.enter_context(tc.tile_pool(name="ytmp", bufs=2))
    po_wp = ctx.enter_context(tc.tile_pool(name="wp", bufs=4))
    po_dram = ctx.enter_context(tc.tile_pool(name="dram_scr", bufs=4,
                                             space="DRAM"))

    mask_sb = po_mask.tile([128, 512], F32, tag="mask")
    nc.sync.dma_start(out=mask_sb[:], in_=cmask[:])
    ident = po_bias.tile([128, 128], F16, tag="ident")
    nc.sync.dma_start(out=ident[:], in_=aps["identin"][:])
    # bva broadcast to all 128 partitions straight from DRAM
    bva_bc = po_bias.tile([128, VW], F32, tag="bva_bc")
    bva2 = aps["bva2"]
    nc.sync.dma_start(out=bva_bc[:], in_=bass.AP(
        tensor=bva2.tensor, offset=bva2.offset,
        ap=[[0, 128]] + [list(a) for a in bva2.ap[1:]]))

    # ---- phase 0: x -> xT via PE transposes ----
    xT = [po_xt.tile([128, T], F16, tag=f"xT{c}", name=f"xT{c}")
          for c in range(CCH)]
    for tt in range(NTT):
        xt_in = po_misc.tile([128, C], F16, tag="miscH", name="xt_in")
        nc.sync.dma_start(out=xt_in[:], in_=x[tt * 128:(tt + 1) * 128, :])
        tsl = slice(tt * 128, (tt + 1) * 128)
        for ca in range(2):
            pst = pp_qk.tile([128, 512], F16, tag="qk", name="pst")
            for j in range(4):
                c = ca * 4 + j
                nc.tensor.transpose(
                    pst[:, j * 128:(j + 1) * 128],
                    xt_in[:, c * 128:(c + 1) * 128],
                    ident[:])
            for j in range(4):
                nc.vector.tensor_copy(
                    xT[ca * 4 + j][:, tsl], pst[:, j * 128:(j + 1) * 128])

    # ---- phase 0b: v (augmented with ones columns, all 8 heads) ----
    v_all = [po_v.tile([128, VW], F16, tag=f"v{tt}", name=f"v{tt}")
             for tt in range(NTT)]
    for half in range(2):
        cs = slice(half * 260, half * 260 + 260)
        wv_sb = []
        for c in range(CCH):
            wt = po_wv.tile([128, 260], F16, tag="wv")
            nc.sync.dma_start(out=wt[:], in_=wva[c * 128:(c + 1) * 128, cs])
            wv_sb.append(wt)
        for tt in range(NTT):
            ps = pp_qk.tile([128, 260], F32, tag="qk")
            for c in range(CCH):
                nc.tensor.matmul(
                    ps[:], xT[c][:, tt * 128:(tt + 1) * 128],
                    wv_sb[c][:], start=(c == 0), stop=(c == CCH - 1))
            nc.vector.tensor_add(v_all[tt][:, cs], ps[:], bva_bc[:, cs])

    # ---- per head pair: qkv -> attention -> partial proj ----
    # Emitted as interleaved work units so the PE instruction stream mixes
    # next-pair qkv (and couple proj) matmuls between attention groups --
    # engines are in-order, so a blocked exp-wait would otherwise stall
    # ready qkv work behind it.

    def prep_qkv(pair):
        psl = slice(pair * 128, (pair + 1) * 128)
        wqk_c = []
        for c in range(CCH):
            wt = po_wqk.tile([128, 256], F16, tag="wqk", name="wt")
            nc.sync.dma_start(out=wt[:, 0:128],
                              in_=wq[c * 128:(c + 1) * 128, psl])
            nc.sync.dma_start(out=wt[:, 128:256],
                              in_=wk[c * 128:(c + 1) * 128, psl])
            wqk_c.append(wt)
        bq_sb = po_bias.tile([128, 1], F32, tag=f"bq{pair}", name=f"bq{pair}")
        nc.sync.dma_start(out=bq_sb[:], in_=bq[psl, :])
        bk_sb = po_bias.tile([128, 1], F32, tag=f"bk{pair}", name=f"bk{pair}")
        nc.sync.dma_start(out=bk_sb[:], in_=bk[psl, :])
        qT = po_qkt.tile([128, T], F16, tag="qT", name="qT")
        kT = po_qkt.tile([128, T], F16, tag="kT", name="kT")
        return dict(wqk=wqk_c, bq=bq_sb, bk=bk_sb, qT=qT, kT=kT)

    def qkv_units(st8):
        units = []
        for tb in range(NTB):
            def unit(tb=tb):
                tsl = slice(tb * 512, (tb + 1) * 512)
                psq = pp_qk.tile([128, 512], F32, tag="qk", name="psq")
                for c in range(CCH):
                    nc.tensor.matmul(psq[:], st8["wqk"][c][:, 0:128],
                                     xT[c][:, tsl],
                                     start=(c == 0), stop=(c == CCH - 1))
                # psum*1/sqrt(D) + bq/sqrt(D)   (bq pre-scaled on host)
                nc.vector.tensor_scalar(
                    out=st8["qT"][:, tsl], in0=psq[:], scalar1=0.125,
                    scalar2=st8["bq"][:], op0=MULT, op1=ADD)
                psk = pp_qk.tile([128, 512], F32, tag="qk", name="psk")
                for c in range(CCH):
                    nc.tensor.matmul(psk[:], st8["wqk"][c][:, 128:256],
                                     xT[c][:, tsl],
                                     start=(c == 0), stop=(c == CCH - 1))
                nc.vector.tensor_scalar(
                    out=st8["kT"][:, tsl], in0=psk[:], scalar1=st8["bk"][:],
                    scalar2=None, op0=ADD)
            units.append(unit)
        return units

    def attn_units(st8, yt):
        qT, kT = st8["qT"], st8["kT"]
        units = []
        for hl in range(2):
            for qb in range(NQB):
                def unit(hl=hl, qb=qb):
                    h = None
                    rq = slice(hl * 64, hl * 64 + 64)
                    pair_h = st8["pair"] * 2 + hl
                    vsl = slice(pair_h * 65, pair_h * 65 + 65)
                    qsl = slice(qb * 512, (qb + 1) * 512)
                    nkt = 4 * qb + 4
                    pv = pp_pv.tile([128, 512], F32, tag="pv", name="pv")

                    def emit_scores(g0, gs, st, offs):
                        for i in range(gs):
                            kt = g0 + i
                            j = kt - 4 * qb
                            off = 128 * j if j > 0 else 0
                            offs.append(off)
                            nc.tensor.matmul(
                                st[:, i * 512 + off:(i + 1) * 512],
                                kT[rq, kt * 128:(kt + 1) * 128],
                                qT[rq, qb * 512 + off:(qb + 1) * 512],
                                start=True, stop=True)
                            if j >= 0:
                                nc.vector.tensor_add(
                                    st[:, i * 512 + off:(i + 1) * 512],
                                    st[:, i * 512 + off:(i + 1) * 512],
                                    mask_sb[:, 0:512 - off])

                    def emit_exp_pv(g0, gs, st, offs):
                        et = po_exp.tile([128, 1024], F16, tag="expT",
                                         name="et")
                        if gs == 2 and offs == [0, 0]:
                            nc.scalar.activation(
                                et[:, 0:1024], st[:, 0:1024],
                                mybir.ActivationFunctionType.Exp)
                        else:
                            for i in range(gs):
                                off = offs[i]
                                nc.scalar.activation(
                                    et[:, i * 512 + off:(i + 1) * 512],
                                    st[:, i * 512 + off:(i + 1) * 512],
                                    mybir.ActivationFunctionType.Exp)
                        for i in range(gs):
                            kt = g0 + i
                            off = offs[i]
                            nc.tensor.matmul(
                                pv[0:65, off:512], v_all[kt][:, vsl],
                                et[:, i * 512 + off:(i + 1) * 512],
                                start=(kt == 0), stop=(kt == nkt - 1))

                    pend = None
                    for g0 in range(0, nkt, 2):
                        gs = min(2, nkt - g0)
                        st = pp_s.tile([128, 1024], F32, tag="s", name="st")
                        offs = []
                        emit_scores(g0, gs, st, offs)
                        if pend is not None:
                            emit_exp_pv(*pend)
                        pend = (g0, gs, st, offs)
                    emit_exp_pv(*pend)
                    # den row 64 -> DRAM bounce broadcast -> recip -> mul
                    den = po_den.tile([128, 512], F32, tag="den", name="den")
                    nc.vector.tensor_copy(den[64:65, :], pv[64:65, :])
                    dscr = po_dram.tile([1, 512], F32, tag="dscr", name="dscr")
                    nc.sync.dma_start(out=dscr[:], in_=den[64:65, :])
                    rec = po_rec.tile([128, 512], F32, tag="recip", name="rec")
                    nc.sync.dma_start(out=rec[0:64, :], in_=bass.AP(
                        tensor=dscr.tensor, offset=dscr[:].offset,
                        ap=[[0, 64]] + [list(a) for a in dscr[:].ap[1:]]))
                    nc.vector.reciprocal_approx_fast(rec[0:64, :],
                                                     rec[0:64, :])
                    if hl == 0:
                        nc.vector.tensor_mul(yt[0:64, qsl], pv[0:64, :],
                                             rec[0:64, :])
                    else:
                        # engines can't cross partitions; bounce via DMA
                        ytmp = po_ytmp.tile([128, 512], F16, tag="ytmp",
                                            name="ytmp")
                        nc.vector.tensor_mul(ytmp[0:64, :], pv[0:64, :],
                                             rec[0:64, :])
                        nc.sync.dma_start(out=yt[64:128, qsl],
                                          in_=ytmp[0:64, :])
                units.append(unit)
        return units

    def prep_proj(couple):
        wp_sb = []
        for pq in range(2):
            for cb in range(2):
                prow = (couple * 2 + pq) * 128
                wt = po_wp.tile([128, 512], F16, tag="wp", name="wpt")
                nc.sync.dma_start(
                    out=wt[:],
                    in_=wp[prow:prow + 128, cb * 512:(cb + 1) * 512])
                wp_sb.append(wt)
        return wp_sb

    def proj_units(couple, wp_sb, yts):
        out_p = out_ab[couple]
        units = []
        for tt0 in range(0, NTT, 2):
            def unit(tt0=tt0):
                for tt in (tt0, tt0 + 1):
                    ot = po_misc.tile([128, C], F32, tag="misc", name="ot")
                    for cb in range(2):
                        ps = pp_qk.tile([128, 512], F32, tag="qk", name="pp")
                        for pq in range(2):
                            nc.tensor.matmul(
                                ps[:],
                                yts[pq][:, tt * 128:(tt + 1) * 128],
                                wp_sb[pq * 2 + cb][:],
                                start=(pq == 0), stop=(pq == 1))
                        nc.vector.tensor_copy(
                            ot[:, cb * 512:(cb + 1) * 512], ps[:])
                    nc.sync.dma_start(
                        out=out_p[tt * 128:(tt + 1) * 128, :], in_=ot[:])
            units.append(unit)
        return units

    def round_robin(*streams):
        streams = [list(s) for s in streams if s]
        while any(streams):
            for s in streams:
                if s:
                    s.pop(0)()

    pair_state = []
    yts = []
    st0 = prep_qkv(0)
    st0["pair"] = 0
    pair_state.append(st0)
    for u in qkv_units(st0):
        u()
    proj_work = {}
    for p in range(NPAIR):
        yt = po_yt.tile([128, T], F16, tag="yT", name="yt")
        yts.append(yt)
        streams = [attn_units(pair_state[p], yt)]
        if p + 1 < NPAIR:
            stn = prep_qkv(p + 1)
            stn["pair"] = p + 1
            pair_state.append(stn)
            streams.append(qkv_units(stn))
        if p == 2:
            wp_sb = prep_proj(0)
            streams.append(proj_units(0, wp_sb, yts[0:2]))
        round_robin(*streams)
    wp_sb = prep_proj(1)
    for u in proj_units(1, wp_sb, yts[2:4]):
        u()

    ctx.close()


_CACHE = {}


def _build():
    if "nc" in _CACHE:
        return _CACHE["nc"]
    nc = bacc.Bacc("TRN2", target_bir_lowering=False, debug=False,
                   enable_asserts=True, num_devices=N_CORES)
    aps = {
        "x": nc.dram_tensor("x", [T, C], F16, kind="ExternalInput").ap(),
        "wq": nc.dram_tensor("wq", [C, F], F16, kind="ExternalInput").ap(),
        "wk": nc.dram_tensor("wk", [C, F], F16, kind="ExternalInput").ap(),
        "wva": nc.dram_tensor("wva", [C, VW], F16, kind="ExternalInput").ap(),
        "bq": nc.dram_tensor("bq", [F, 1], F32, kind="ExternalInput").ap(),
        "bk": nc.dram_tensor("bk", [F, 1], F32, kind="ExternalInput").ap(),
        "bva2": nc.dram_tensor("bva2", [1, VW], F32, kind="ExternalInput").ap(),
        "wp": nc.dram_tensor("wp", [F, C], F16, kind="ExternalInput").ap(),
        "cmask": nc.dram_tensor("cmask", [128, 512], F32,
                                kind="ExternalInput").ap(),
        "identin": nc.dram_tensor("identin", [128, 128], F16,
                                  kind="ExternalInput").ap(),
        "out_pa": nc.dram_tensor("out_pa", [T, C], F32,
                                 kind="ExternalOutput").ap(),
        "out_pb": nc.dram_tensor("out_pb", [T, C], F32,
                                 kind="ExternalOutput").ap(),
    }
    with tile.TileContext(nc) as tc:
        _emit(tc, aps)
    nc.compile()
    _CACHE["nc"] = nc
    return nc


def _make_in_maps(x, Wqkv, bqkv, Wproj):
    x = np.asarray(x, dtype=np.float32)
    Wqkv = np.asarray(Wqkv, dtype=np.float32)
    bqkv = np.asarray(bqkv, dtype=np.float32)
    Wproj = np.asarray(Wproj, dtype=np.float32)

    # triangular causal mask: M[p, f] = 0 if f >= p else -1e9
    p_idx = np.arange(128)[:, None]
    u_idx = np.arange(512)[None, :]
    cmask = np.where(u_idx >= p_idx, 0.0, -1e9).astype(np.float32)

    in_maps = []
    for core in range(N_CORES):
        b, g = divmod(core, 2)
        q0, k0, v0 = 512 * g, C + 512 * g, 2 * C + 512 * g
        wva = np.zeros((C, VW), dtype=np.float32)
        bva = np.zeros((1, VW), dtype=np.float32)
        for h in range(NH):
            src = v0 + D * h
            dst = 65 * h
            # per-head layout [v(64), one]
            wva[:, dst:dst + 64] = Wqkv[:, src:src + 64]
            bva[0, dst:dst + 64] = bqkv[src:src + 64]
            bva[0, dst + 64] = 1.0
        in_maps.append({
            "x": np.ascontiguousarray(x[b]).astype(np.float16),
            "wq": np.ascontiguousarray(Wqkv[:, q0:q0 + F]).astype(np.float16),
            "wk": np.ascontiguousarray(Wqkv[:, k0:k0 + F]).astype(np.float16),
            "wva": wva.astype(np.float16),
            "bq": np.ascontiguousarray(bqkv[q0:q0 + F].reshape(F, 1) * 0.125),
            "bk": np.ascontiguousarray(bqkv[k0:k0 + F].reshape(F, 1)),
            "bva2": bva,
            "wp": np.ascontiguousarray(Wproj[512 * g:512 * g + F, :]).astype(np.float16),
            "cmask": cmask,
            "identin": np.eye(128, dtype=np.float16),
        })
    return in_maps


def run_sharded(x, Wqkv, bqkv, Wproj, bproj, trace=False):
    nc = _build()
    in_maps = _make_in_maps(x, Wqkv, bqkv, Wproj)
    res = run_bass_kernel_spmd(nc, in_maps, core_ids=list(range(N_CORES)),
                               trace=trace)
    bproj = np.asarray(bproj, dtype=np.float32)
    out = np.empty((B, T, C), dtype=np.float32)
    for b in range(B):
        acc = bproj[None, :].astype(np.float32).repeat(T, axis=0)
        for core in (2 * b, 2 * b + 1):
            acc = acc + res.results[core]["out_pa"] + res.results[core]["out_pb"]
        out[b] = acc
    return out, res


def kernel(x, Wqkv, bqkv, Wproj, bproj):
    out, _ = run_sharded(x, Wqkv, bqkv, Wproj, bproj, trace=False)
    return out
